# revision 11
# baseline (speedup 1.0000x reference)
"""GAT layer (global-softmax variant) as a 2-launch Bass kernel on 8 trn2 cores.

Math (the reference einsum 'hid,nf->hnd' has no shared index, so it factors):
    xsum[n]  = sum_f x[n, f]
    wsum[h,d]= sum_i W[h, i, d]            -> Wh[h,n,d] = wsum[h,d] * xsum[n]
    cs[h]    = sum_d wsum[h,d] * a[h, d, 0]
    cd[h]    = sum_d wsum[h,d] * a[h, D+d, 0]
    e[h,k]   = leakyrelu(cs[h]*xsum[src_k] + cd[h]*xsum[dst_k], 0.2)
    alpha    = softmax over all E edges (per head)
    t[h,n]   = sum_{k: dst_k = n} xsum[src_k] * alpha[h,k]
    out[n, h*D+d] = elu(wsum[h,d] * t[h,n] + bias[h,d])

Sharding: nodes (and their incoming edges) are sharded across 8 cores; the
host additionally sorts each core's nodes by in-degree (descending) and
builds two dense degree-padded grids of gathered xsum[src] values: a base
band of width W0 covering every node and a narrow spill band covering only
the first nb1 blocks (the high-degree nodes).  The device then runs only
dense passes: a fused DVE op computes leakyrelu(cs*U + cd*V), ACT
exponentiates (with a per-head shift M; softmax is shift-invariant), a
bf16 multiply + block-reduce yields t, and an 8-float AllReduce combines
the softmax denominators.  Pad slots hold u=0 so they add 0 to t; their
exp(lrelu(cd*v)-M) contribution to Z is removed analytically via the
per-node pad-count correction.  All tensors cross PCIe pre-swizzled into
the device's [partition, block] layout so every DMA is contiguous.
"""

import os
import sys

for _p in ("/opt/trn_rl_repo", "/root/.axon_site/_ro/trn_rl_repo"):
    if os.path.isdir(_p) and _p not in sys.path:
        sys.path.insert(0, _p)

import dataclasses
import functools

import numpy as np
import ml_dtypes

import concourse.bass as bass
import concourse.tile as tile
from concourse import bacc, mybir
from concourse.bass_utils import run_bass_kernel_spmd

F32 = mybir.dt.float32
BF16 = mybir.dt.bfloat16
ALU = mybir.AluOpType
ACTF = mybir.ActivationFunctionType

NCORES = 8
N = 100000
E = 1600000
H = 8
D = 8
F_IN = 128
NPC = N // NCORES          # 12500 real nodes per core
B = 98                     # blocks per partition (128 * 98 = 12544)
PN = 128 * B               # padded nodes per core (12544)

# bf16 for the u*p multiply + block reduce (t only; Z stays f32)
USE_BF16_PROD = bool(int(os.environ.get("GAT_BF16", "1")))

# Populated by kernel() for test harnesses to inspect.
LAST_STATS = {}

_TRACE = bool(int(os.environ.get("GAT_TRACE", "0")))

# --------------------------------------------------------------------------
# custom DVE op: out = leakyrelu(in0*s0 + in1*s1) with slope (1 - imm2)
#   z = Src0*C0 + Src1*C1 ; out = z - min(z, 0)*C2      (imm2 = 0.8 -> 0.2)
# --------------------------------------------------------------------------


def _comb_ref(in0, in1, s0, s1, imm2):
    z = in0.astype(np.float32) * s0 + in1.astype(np.float32) * s1
    return (z - np.minimum(z, 0.0) * imm2).astype(np.float32)


@functools.cache
def _register_comb_op():
    import concourse.dve_ops as dve_ops
    from concourse.dve_spec import Spec, Src0, Src1, C0, C1, C2, Zero, minn, lower
    from concourse.dve_uop import DveOpSpec

    name = "GAT_COMB_LRELU"
    if name in dve_ops._SUB_OPCODE_FOR_NAME:
        return next(op for op in dve_ops.OPS if op.name == name)

    z = Src0 * C0 + Src1 * C1
    spec = Spec(body=z - minn(z, Zero) * C2, reference=_comb_ref)

    row = dve_ops._CUSTOM_DVE_ROW_BASE + len(dve_ops.OPS)
    assert row < 0x20
    shas = {}
    for ver in ("v3", "v4"):
        s = DveOpSpec(name=name, opcode=row, uops=lower(spec, ver=ver), rd1_en=True)
        shas[ver] = s.sha(ver)
    op = dve_ops.DveOp(name, spec, subdim=False, uops_sha=shas)
    dve_ops.OPS.append(op)
    dve_ops.CUSTOM_DVE_SPECS[name] = spec
    dve_ops._SUB_OPCODE_FOR_NAME[name] = row
    return op


def _elu_tail_ref(in0, in1, s0, s1, imm2):
    return (np.maximum(in0.astype(np.float32), 0.0) + in1 - 1.0).astype(np.float32)


@functools.cache
def _register_elu_tail_op():
    import concourse.dve_ops as dve_ops
    from concourse.dve_spec import Spec, Src0, Src1, Zero, One, maxx, lower
    from concourse.dve_uop import DveOpSpec

    name = "GAT_ELU_TAIL"
    if name in dve_ops._SUB_OPCODE_FOR_NAME:
        return next(op for op in dve_ops.OPS if op.name == name)

    spec = Spec(body=maxx(Src0, Zero) + Src1 - One, reference=_elu_tail_ref)
    row = dve_ops._CUSTOM_DVE_ROW_BASE + len(dve_ops.OPS)
    assert row < 0x20
    shas = {}
    for ver in ("v3", "v4"):
        sp = DveOpSpec(name=name, opcode=row, uops=lower(spec, ver=ver), rd1_en=True)
        shas[ver] = sp.sha(ver)
    op = dve_ops.DveOp(name, spec, subdim=False, uops_sha=shas)
    dve_ops.OPS.append(op)
    dve_ops.CUSTOM_DVE_SPECS[name] = spec
    dve_ops._SUB_OPCODE_FOR_NAME[name] = row
    return op


def _ap_with(ap: bass.AP, dims, offset_elems=0):
    """Clone `ap` with explicit [step, count] dims (element units).

    dims[0] is the partition dim; pass None to keep the AP's own.
    """
    dims = [list(ap.ap[0]) if d is None else list(d) for d in dims]
    return dataclasses.replace(ap, ap=dims, offset=ap.offset + offset_elems)


# --------------------------------------------------------------------------
# launch 1: xsum over features (per core: x slice [PN, 128] -> xsum [PN])
# --------------------------------------------------------------------------


@functools.cache
def _build_nc1():
    nc = bacc.Bacc("TRN2", target_bir_lowering=False, debug=False)
    x_d = nc.dram_tensor("x", [PN, F_IN], F32, kind="ExternalInput")
    xs_d = nc.dram_tensor("xsum", [PN], F32, kind="ExternalOutput")

    x_ap = x_d.ap().rearrange("(p b) f -> p b f", p=128)       # [128, 98, 128]
    xs_ap = xs_d.ap().rearrange("(p b) -> p b", p=128)         # [128, 98]

    CH = 14
    BPC = B // CH
    with tile.TileContext(nc) as tc:
        with (
            tc.tile_pool(name="xin", bufs=3) as xin_pool,
            tc.tile_pool(name="xs", bufs=1) as xs_pool,
        ):
            xs_t = xs_pool.tile([128, B], F32)
            for ch in range(CH):
                xt = xin_pool.tile([128, BPC, F_IN], F32)
                nc.sync.dma_start(out=xt[:], in_=x_ap[:, ch * BPC:(ch + 1) * BPC, :])
                nc.vector.tensor_reduce(
                    out=xs_t[:, ch * BPC:(ch + 1) * BPC],
                    in_=xt[:],
                    axis=mybir.AxisListType.X,
                    op=ALU.add,
                )
            nc.sync.dma_start(out=xs_ap, in_=xs_t[:])
    nc.compile()
    return nc


# --------------------------------------------------------------------------
# launch 2: the GAT edge math on dense degree-banded grids
# --------------------------------------------------------------------------


@functools.cache
def _build_nc2(w0: int, w1: int, nb1: int):
    comb = _register_comb_op()
    elu_tail = _register_elu_tail_op()
    FB0 = B * w0
    FB1 = nb1 * w1
    PDT = BF16 if USE_BF16_PROD else F32

    nc = bacc.Bacc("TRN2", target_bir_lowering=False, debug=False, num_devices=NCORES)
    # all grid inputs arrive pre-swizzled: [128 partitions, per-partition data]
    v_d = nc.dram_tensor("v", [128, B], F32, kind="ExternalInput")
    pc_d = nc.dram_tensor("pc", [128, B], F32, kind="ExternalInput")
    # br = [cs(8) cd(8) -M(8) exp(-M)(8)]: weight-derived scalars, host-folded
    br_d = nc.dram_tensor("br", [1, 32], F32, kind="ExternalInput")
    ws_d = nc.dram_tensor("wsvec", [1, H * D], F32, kind="ExternalInput")
    b_d = nc.dram_tensor("bvec", [1, H * D], F32, kind="ExternalInput")
    u0_d = nc.dram_tensor("U0", [128, FB0], F32, kind="ExternalInput")
    u016_d = nc.dram_tensor("U0_16", [128, FB0], PDT, kind="ExternalInput")
    if nb1:
        u1_d = nc.dram_tensor("U1", [128, FB1], F32, kind="ExternalInput")
        u116_d = nc.dram_tensor("U1_16", [128, FB1], PDT, kind="ExternalInput")
    out_d = nc.dram_tensor("out", [128, B * H * D], F32, kind="ExternalOutput")

    zin_d = nc.dram_tensor("zin", [1, H], F32)
    zout_d = nc.dram_tensor("zout", [1, H], F32, addr_space="Shared")

    with tile.TileContext(nc) as tc:
        with (
            tc.tile_pool(name="singles", bufs=1) as singles,
            tc.tile_pool(name="work", bufs=2) as work,
            tc.tile_pool(name="small", bufs=2) as small,
            tc.tile_pool(name="psum", bufs=2, space="PSUM") as psum,
        ):
            # ---- small loads first ----
            V1 = singles.tile([128, B], F32)
            nc.sync.dma_start(out=V1[:], in_=v_d.ap())
            PC1 = singles.tile([128, B], F32)
            nc.sync.dma_start(out=PC1[:], in_=pc_d.ap())
            BR = singles.tile([128, 32], F32)
            nc.sync.dma_start(out=BR[:], in_=_ap_with(br_d.ap(), [[0, 128], [1, 32]]))
            ws_row = singles.tile([1, H * D], F32)
            nc.sync.dma_start(out=ws_row[:], in_=ws_d.ap())
            bias_t = singles.tile([1, H * D], F32)
            nc.sync.dma_start(out=bias_t[:], in_=b_d.ap())
            # ---- big grid loads ----
            U0 = singles.tile([128, FB0], F32)
            nc.sync.dma_start(out=U0[:], in_=u0_d.ap())
            U016 = singles.tile([128, FB0], PDT)
            nc.sync.dma_start(out=U016[:], in_=u016_d.ap())
            if nb1:
                U1 = singles.tile([128, FB1], F32)
                nc.sync.dma_start(out=U1[:], in_=u1_d.ap())
                U116 = singles.tile([128, FB1], PDT)
                nc.sync.dma_start(out=U116[:], in_=u116_d.ap())

            ones_col = singles.tile([128, 1], F32)
            nc.vector.memset(ones_col[:], 1.0)
            ones_row = singles.tile([1, 128], F32)
            nc.vector.memset(ones_row[:], 1.0)

            # ---- V broadcast along occ for both bands ----
            VB0 = singles.tile([128, FB0], F32)
            nc.vector.tensor_copy(
                out=VB0[:], in_=_ap_with(V1[:], [None, [1, B], [0, w0]]),
            )
            if nb1:
                VB1 = singles.tile([128, FB1], F32)
                nc.vector.tensor_copy(
                    out=VB1[:], in_=_ap_with(V1[:], [None, [1, nb1], [0, w1]]),
                )

            # ---- batched pad correction ----
            # padz[:, h] = em[h] * sum_b pc[b] * exp(lrelu(cd[h]*v[b]))
            padz = singles.tile([128, H], F32)
            cdv = small.tile([128, H, B], F32, tag="cdv")
            nc.vector.tensor_tensor(
                out=cdv[:],
                in0=_ap_with(V1[:], [None, [0, H], [1, B]]),
                in1=_ap_with(BR[:], [None, [1, H], [0, B]], offset_elems=8),
                op=ALU.mult,
            )
            cdm = small.tile([128, H, B], F32, tag="cdm")
            nc.vector.tensor_scalar_min(cdm[:], cdv[:], 0.0)
            nc.vector.tensor_scalar(
                out=cdm[:], in0=cdm[:], scalar1=-0.8, scalar2=None, op0=ALU.mult,
            )
            nc.vector.tensor_add(cdm[:], cdm[:], cdv[:])
            pexp = small.tile([128, H, B], F32, tag="pexp")
            nc.scalar.activation(pexp[:], cdm[:], ACTF.Exp)
            pw = small.tile([128, H, B], F32, tag="pw")
            nc.vector.tensor_tensor(
                out=pw[:],
                in0=_ap_with(PC1[:], [None, [0, H], [1, B]]),
                in1=_ap_with(BR[:], [None, [1, H], [0, B]], offset_elems=24),
                op=ALU.mult,
            )
            nc.vector.tensor_mul(pexp[:], pexp[:], pw[:])
            nc.vector.tensor_reduce(
                out=padz[:], in_=pexp[:], axis=mybir.AxisListType.X, op=ALU.add,
            )

            # ---- phase 1: exp grids for all heads (Z ready early) ----
            zgrid = singles.tile([128, 2 * H], F32)   # [band0 | band1]
            T = singles.tile([128, H, B], F32)
            p0s = singles.tile([128, H, FB0], PDT, name="p0s")
            p1s = singles.tile([128, H, FB1], PDT, name="p1s") if nb1 else None

            for h in range(H):
                w0_t = work.tile([128, FB0], F32, tag="w0", name=f"w0t{h}")
                nc.vector._custom_dve(
                    comb, out=w0_t[:], in0=U0[:], in1=VB0[:],
                    s0=BR[:, h:h + 1], s1=BR[:, 8 + h:9 + h], imm2=0.8,
                )
                nc.scalar.activation(
                    p0s[:, h, :], w0_t[:], ACTF.Exp,
                    bias=BR[:, 16 + h:17 + h], scale=1.0,
                    accum_out=zgrid[:, h:h + 1],
                )
                if nb1:
                    w1_t = work.tile([128, FB1], F32, tag="w1", name=f"w1t{h}")
                    nc.vector._custom_dve(
                        comb, out=w1_t[:], in0=U1[:], in1=VB1[:],
                        s0=BR[:, h:h + 1], s1=BR[:, 8 + h:9 + h], imm2=0.8,
                    )
                    nc.scalar.activation(
                        p1s[:, h, :], w1_t[:], ACTF.Exp,
                        bias=BR[:, 16 + h:17 + h], scale=1.0,
                        accum_out=zgrid[:, H + h:H + h + 1],
                    )

            # ---- Z partials out + AllReduce (overlaps phase 2) ----
            zv = singles.tile([128, H], F32)
            if nb1:
                nc.vector.tensor_add(zv[:], zgrid[:, 0:H], zgrid[:, H:2 * H])
                nc.vector.tensor_sub(zv[:], zv[:], padz[:])
            else:
                nc.vector.tensor_sub(zv[:], zgrid[:, 0:H], padz[:])
            psum_z = psum.tile([1, H], F32)
            nc.tensor.matmul(psum_z[:], ones_col[:], zv[:])
            zr = singles.tile([1, H], F32)
            nc.vector.tensor_copy(zr[:], psum_z[:])
            nc.sync.dma_start(out=zin_d.ap(), in_=zr[:])
            nc.gpsimd.collective_compute(
                "AllReduce",
                ALU.add,
                replica_groups=[list(range(NCORES))],
                ins=[zin_d.ap()],
                outs=[zout_d.ap()],
            )

            # ---- phase 2: t = sum_j u * p per block (overlaps the AR) ----
            for h in range(H):
                prod0 = work.tile([128, FB0], PDT, tag="prod0", name=f"pr0t{h}")
                nc.vector.tensor_mul(prod0[:], U016[:], p0s[:, h, :])
                nc.vector.tensor_reduce(
                    out=T[:, h, :],
                    in_=prod0[:].rearrange("p (b j) -> p b j", b=B),
                    axis=mybir.AxisListType.X,
                    op=ALU.add,
                )
                if nb1:
                    prod1 = work.tile([128, FB1], PDT, tag="prod1", name=f"pr1t{h}")
                    nc.vector.tensor_mul(prod1[:], U116[:], p1s[:, h, :])
                    t1 = small.tile([128, nb1], F32, tag="t1", name=f"t1t{h}")
                    nc.vector.tensor_reduce(
                        out=t1[:],
                        in_=prod1[:].rearrange("p (b j) -> p b j", b=nb1),
                        axis=mybir.AxisListType.X,
                        op=ALU.add,
                    )
                    nc.vector.tensor_add(T[:, h, 0:nb1], T[:, h, 0:nb1], t1[:])

            zr2 = singles.tile([1, H], F32)
            nc.sync.dma_start(out=zr2[:], in_=zout_d.ap())
            zinv = singles.tile([1, H], F32)
            nc.vector.reciprocal(zinv[:], zr2[:])

            # wz[h*8+d] = ws[h,d]*zinv[h]; pack [wz(64) bias(64)], bcast via PE
            wzrow = singles.tile([1, 128], F32)
            nc.vector.tensor_tensor(
                out=wzrow[:, 0:64].rearrange("p (h d) -> p h d", h=H),
                in0=ws_row[:].rearrange("p (h d) -> p h d", h=H),
                in1=_ap_with(zinv[:], [None, [1, H], [0, D]]),
                op=ALU.mult,
            )
            nc.vector.tensor_copy(wzrow[:, 64:128], bias_t[:])
            psum_wb = psum.tile([128, 128], F32)
            nc.tensor.matmul(psum_wb[:], ones_row[:], wzrow[:])
            WB = singles.tile([128, 128], F32)
            nc.vector.tensor_copy(WB[:], psum_wb[:])

            # ---- epilogue: out = elu(t*wz + bias), 7 chunks of 14 blocks ----
            # engine split: DVE (y-mult, max-add), ACT (relu(-s), exp(-r)),
            # GPSIMD (bias-add, final add)
            CB = 14
            t_pstep = T[:].ap[0][0]
            wb_pstep = WB[:].ap[0][0]
            out_ap = out_d.ap().rearrange("p (b c) -> p b c", c=H * D)
            with (
                tc.tile_pool(name="epi", bufs=2) as epi,
            ):
                for ch in range(B // CB):
                    y_t = epi.tile([128, CB, H * D], F32, tag="y")
                    # y[p, b, h*8+d] = T[p, h, ch*CB+b] * WB[p, h*8+d]
                    nc.vector.tensor_tensor(
                        out=y_t[:],
                        in0=_ap_with(T[:], [[t_pstep, 128], [1, CB], [B, H], [0, D]],
                                     offset_elems=ch * CB),
                        in1=_ap_with(WB[:], [[wb_pstep, 128], [0, CB], [1, H * D]]),
                        op=ALU.mult,
                    )
                    s_t = epi.tile([128, CB, H * D], F32, tag="s")
                    nc.vector.tensor_tensor(
                        out=s_t[:],
                        in0=y_t[:],
                        in1=_ap_with(WB[:], [[wb_pstep, 128], [0, CB], [1, H * D]],
                                     offset_elems=64),
                        op=ALU.add,
                    )
                    # r = relu(-s) = -min(s,0);  q = exp(-r) = exp(min(s,0))
                    r_t = epi.tile([128, CB, H * D], F32, tag="r")
                    nc.scalar.activation(r_t[:], s_t[:], ACTF.Relu, scale=-1.0)
                    q_t = epi.tile([128, CB, H * D], F32, tag="q")
                    nc.scalar.activation(q_t[:], r_t[:], ACTF.Exp, scale=-1.0)
                    o_t = epi.tile([128, CB, H * D], F32, tag="o")
                    # elu(s) = max(s,0) + exp(min(s,0)) - 1
                    nc.vector._custom_dve(
                        elu_tail, out=o_t[:].rearrange("p b c -> p (b c)"),
                        in0=s_t[:].rearrange("p b c -> p (b c)"),
                        in1=q_t[:].rearrange("p b c -> p (b c)"),
                    )
                    nc.sync.dma_start(
                        out=out_ap[:, ch * CB:(ch + 1) * CB, :], in_=o_t[:],
                    )
    nc.compile()
    return nc


# --------------------------------------------------------------------------
# host orchestration
# --------------------------------------------------------------------------


def _swz(arr):
    """[PN, ...] row-major (block-major node ids: ln = b*128 + p) ->
    device layout [128, B, ...] flattened per partition."""
    rest = arr.shape[1:]
    return np.ascontiguousarray(
        arr.reshape(B, 128, *rest).transpose(1, 0, *range(2, 2 + len(rest)))
    ).reshape(128, -1)


def kernel(x, edge_index, W, a, bias):
    x = np.ascontiguousarray(np.asarray(x), dtype=np.float32)
    ei = np.asarray(edge_index)
    src = ei[0].astype(np.int64)
    dst = ei[1].astype(np.int64)
    W = np.asarray(W, dtype=np.float32)
    a = np.asarray(a, dtype=np.float32)
    bias = np.asarray(bias, dtype=np.float32)

    cores = list(range(NCORES))

    # ---- launch 1: xsum ----
    nc1 = _build_nc1()
    in1 = []
    for c in cores:
        xp = np.zeros((PN, F_IN), dtype=np.float32)
        xp[:NPC] = x[c * NPC:(c + 1) * NPC]
        in1.append({"x": xp})
    r1 = run_bass_kernel_spmd(nc1, in1, cores, trace=_TRACE)
    xsum = np.concatenate([r1.results[c]["xsum"][:NPC] for c in cores])
    LAST_STATS["launch1_ns"] = r1.exec_time_ns

    gmax = float(np.abs(xsum).max())

    # ---- host: degree-sorted banded grids (index work + one gather) ----
    deg = np.bincount(dst, minlength=N)
    degc = deg.reshape(NCORES, NPC)
    dmax = int(deg.max())

    # pick band split minimizing total slots
    best = None
    for cand in range(8, dmax + 1):
        cnt = int((degc > cand).sum(1).max())
        nb = -(-cnt // 128) if cnt else 0
        slots = PN * cand + nb * 128 * (dmax - cand)
        if best is None or slots < best[0]:
            best = (slots, cand, nb)
    _, W0, nb1 = best
    W1 = dmax - W0 if nb1 else 0

    # per-core degree-descending node order
    deg_pad = np.zeros((NCORES, PN), dtype=np.int64)
    deg_pad[:, :NPC] = degc
    perm = np.argsort(-deg_pad, axis=1, kind="stable")       # [NC, PN]
    inv_perm = np.argsort(perm, axis=1)                      # orig ln -> sorted row

    # per-edge target slots
    order = np.argsort(dst, kind="stable")
    sdst = dst[order]
    usrc = xsum[src[order]]
    starts = np.concatenate([[0], np.cumsum(deg)[:-1]])
    occ = np.arange(E, dtype=np.int64) - starts[sdst]
    core_idx = sdst // NPC
    row = inv_perm[core_idx, sdst % NPC]                     # sorted row id

    U0_all = np.zeros((NCORES, PN, W0), dtype=np.float32)
    m0 = occ < W0
    U0_all[core_idx[m0], row[m0], occ[m0]] = usrc[m0]
    if nb1:
        U1_all = np.zeros((NCORES, nb1 * 128, W1), dtype=np.float32)
        m1 = ~m0
        U1_all[core_idx[m1], row[m1], occ[m1] - W0] = usrc[m1]

    cap = np.full(PN, W0, dtype=np.float32)
    cap[:nb1 * 128] += W1
    sdeg = np.take_along_axis(deg_pad, perm, axis=1).astype(np.float32)
    pc_all = cap[None, :] - sdeg                             # [NC, PN]

    v_all = np.zeros((NCORES, PN), dtype=np.float32)
    v_all[:, :NPC] = xsum.reshape(NCORES, NPC)
    v_all = np.take_along_axis(v_all, perm, axis=1)

    pdt = ml_dtypes.bfloat16 if USE_BF16_PROD else np.float32

    # weight-derived scalars (pure weight preprocessing)
    wsum = W.sum(1)                                 # [H, D]
    cs = (wsum * a[:, :D, 0]).sum(1)                # [H]
    cd = (wsum * a[:, D:, 0]).sum(1)
    M = (np.abs(cs) + np.abs(cd)) * gmax - 40.0
    brvec = np.concatenate([cs, cd, -M, np.exp(-M)]).astype(np.float32)[None, :]
    wsvec = np.ascontiguousarray(wsum.reshape(1, H * D)).astype(np.float32)
    bvec = np.ascontiguousarray(bias.reshape(1, H * D))

    # ---- launch 2 ----
    nc2 = _build_nc2(W0, W1, nb1)
    in2 = []
    for c in cores:
        m = {
            "v": _swz(v_all[c][:, None]),
            "pc": _swz(pc_all[c][:, None]),
            "br": brvec,
            "wsvec": wsvec,
            "bvec": bvec,
            "U0": _swz(U0_all[c]),
        }
        m["U0_16"] = m["U0"].astype(pdt)
        if nb1:
            u1 = np.ascontiguousarray(
                U1_all[c].reshape(nb1, 128, W1).transpose(1, 0, 2)
            ).reshape(128, -1)
            m["U1"] = u1
            m["U1_16"] = u1.astype(pdt)
        in2.append(m)
    r2 = run_bass_kernel_spmd(nc2, in2, cores, trace=_TRACE)
    LAST_STATS["launch2_ns"] = r2.exec_time_ns

    out = np.empty((N, H * D), dtype=np.float32)
    for c in cores:
        o = r2.results[c]["out"].reshape(128, B, H * D).transpose(1, 0, 2)
        o = o.reshape(PN, H * D)          # rows in sorted order
        sel = perm[c] < NPC
        out[c * NPC + perm[c][sel]] = o[sel]
    return out


# revision 12
# speedup vs baseline: 1.0456x; 1.0456x over previous
"""GAT layer (global-softmax variant) as a 2-launch Bass kernel on 8 trn2 cores.

Math (the reference einsum 'hid,nf->hnd' has no shared index, so it factors):
    xsum[n]  = sum_f x[n, f]
    wsum[h,d]= sum_i W[h, i, d]            -> Wh[h,n,d] = wsum[h,d] * xsum[n]
    cs[h]    = sum_d wsum[h,d] * a[h, d, 0]
    cd[h]    = sum_d wsum[h,d] * a[h, D+d, 0]
    e[h,k]   = leakyrelu(cs[h]*xsum[src_k] + cd[h]*xsum[dst_k], 0.2)
    alpha    = softmax over all E edges (per head)
    t[h,n]   = sum_{k: dst_k = n} xsum[src_k] * alpha[h,k]
    out[n, h*D+d] = elu(wsum[h,d] * t[h,n] + bias[h,d])

Sharding: nodes (and their incoming edges) are sharded across 8 cores; the
host additionally sorts each core's nodes by in-degree (descending) and
builds two dense degree-padded grids of gathered xsum[src] values: a base
band of width W0 covering every node and a narrow spill band covering only
the first nb1 blocks (the high-degree nodes).  The device then runs only
dense passes: a fused DVE op computes leakyrelu(cs*U + cd*V), ACT
exponentiates (with a per-head shift M; softmax is shift-invariant), a
bf16 multiply + block-reduce yields t, and an 8-float AllReduce combines
the softmax denominators.  Pad slots hold u=0 so they add 0 to t; their
exp(lrelu(cd*v)-M) contribution to Z is removed analytically via the
per-node pad-count correction.  All tensors cross PCIe pre-swizzled into
the device's [partition, block] layout so every DMA is contiguous.
"""

import os
import sys

for _p in ("/opt/trn_rl_repo", "/root/.axon_site/_ro/trn_rl_repo"):
    if os.path.isdir(_p) and _p not in sys.path:
        sys.path.insert(0, _p)

import dataclasses
import functools

import numpy as np
import ml_dtypes

import concourse.bass as bass
import concourse.tile as tile
from concourse import bacc, mybir
from concourse.bass_utils import run_bass_kernel_spmd

F32 = mybir.dt.float32
BF16 = mybir.dt.bfloat16
ALU = mybir.AluOpType
ACTF = mybir.ActivationFunctionType

NCORES = 8
N = 100000
E = 1600000
H = 8
D = 8
F_IN = 128
NPC = N // NCORES          # 12500 real nodes per core
B = 98                     # blocks per partition (128 * 98 = 12544)
PN = 128 * B               # padded nodes per core (12544)

# bf16 for the u*p multiply + block reduce (t only; Z stays f32)
USE_BF16_PROD = bool(int(os.environ.get("GAT_BF16", "1")))

# Populated by kernel() for test harnesses to inspect.
LAST_STATS = {}

_TRACE = bool(int(os.environ.get("GAT_TRACE", "0")))

# --------------------------------------------------------------------------
# custom DVE op: out = leakyrelu(in0*s0 + in1*s1) with slope (1 - imm2)
#   z = Src0*C0 + Src1*C1 ; out = z - min(z, 0)*C2      (imm2 = 0.8 -> 0.2)
# --------------------------------------------------------------------------


def _comb_ref(in0, in1, s0, s1, imm2):
    z = in0.astype(np.float32) * s0 + in1.astype(np.float32) * s1
    return (z - np.minimum(z, 0.0) * imm2).astype(np.float32)


@functools.cache
def _register_comb_op():
    import concourse.dve_ops as dve_ops
    from concourse.dve_spec import Spec, Src0, Src1, C0, C1, C2, Zero, minn, lower
    from concourse.dve_uop import DveOpSpec

    name = "GAT_COMB_LRELU"
    if name in dve_ops._SUB_OPCODE_FOR_NAME:
        return next(op for op in dve_ops.OPS if op.name == name)

    z = Src0 * C0 + Src1 * C1
    spec = Spec(body=z - minn(z, Zero) * C2, reference=_comb_ref)

    row = dve_ops._CUSTOM_DVE_ROW_BASE + len(dve_ops.OPS)
    assert row < 0x20
    shas = {}
    for ver in ("v3", "v4"):
        s = DveOpSpec(name=name, opcode=row, uops=lower(spec, ver=ver), rd1_en=True)
        shas[ver] = s.sha(ver)
    op = dve_ops.DveOp(name, spec, subdim=False, uops_sha=shas)
    dve_ops.OPS.append(op)
    dve_ops.CUSTOM_DVE_SPECS[name] = spec
    dve_ops._SUB_OPCODE_FOR_NAME[name] = row
    return op


def _elu_tail_ref(in0, in1, s0, s1, imm2):
    return (np.maximum(in0.astype(np.float32), 0.0) + in1 - 1.0).astype(np.float32)


@functools.cache
def _register_elu_tail_op():
    import concourse.dve_ops as dve_ops
    from concourse.dve_spec import Spec, Src0, Src1, Zero, One, maxx, lower
    from concourse.dve_uop import DveOpSpec

    name = "GAT_ELU_TAIL"
    if name in dve_ops._SUB_OPCODE_FOR_NAME:
        return next(op for op in dve_ops.OPS if op.name == name)

    spec = Spec(body=maxx(Src0, Zero) + Src1 - One, reference=_elu_tail_ref)
    row = dve_ops._CUSTOM_DVE_ROW_BASE + len(dve_ops.OPS)
    assert row < 0x20
    shas = {}
    for ver in ("v3", "v4"):
        sp = DveOpSpec(name=name, opcode=row, uops=lower(spec, ver=ver), rd1_en=True)
        shas[ver] = sp.sha(ver)
    op = dve_ops.DveOp(name, spec, subdim=False, uops_sha=shas)
    dve_ops.OPS.append(op)
    dve_ops.CUSTOM_DVE_SPECS[name] = spec
    dve_ops._SUB_OPCODE_FOR_NAME[name] = row
    return op


def _ap_with(ap: bass.AP, dims, offset_elems=0):
    """Clone `ap` with explicit [step, count] dims (element units).

    dims[0] is the partition dim; pass None to keep the AP's own.
    """
    dims = [list(ap.ap[0]) if d is None else list(d) for d in dims]
    return dataclasses.replace(ap, ap=dims, offset=ap.offset + offset_elems)


# --------------------------------------------------------------------------
# launch 1: xsum over features (per core: x slice [PN, 128] -> xsum [PN])
# --------------------------------------------------------------------------


@functools.cache
def _build_nc1():
    nc = bacc.Bacc("TRN2", target_bir_lowering=False, debug=False)
    x_d = nc.dram_tensor("x", [PN, F_IN], F32, kind="ExternalInput")
    xs_d = nc.dram_tensor("xsum", [PN], F32, kind="ExternalOutput")

    x_ap = x_d.ap().rearrange("(p b) f -> p b f", p=128)       # [128, 98, 128]
    xs_ap = xs_d.ap().rearrange("(p b) -> p b", p=128)         # [128, 98]

    CH = 14
    BPC = B // CH
    with tile.TileContext(nc) as tc:
        with (
            tc.tile_pool(name="xin", bufs=3) as xin_pool,
            tc.tile_pool(name="xs", bufs=1) as xs_pool,
        ):
            xs_t = xs_pool.tile([128, B], F32)
            for ch in range(CH):
                xt = xin_pool.tile([128, BPC, F_IN], F32)
                nc.sync.dma_start(out=xt[:], in_=x_ap[:, ch * BPC:(ch + 1) * BPC, :])
                nc.vector.tensor_reduce(
                    out=xs_t[:, ch * BPC:(ch + 1) * BPC],
                    in_=xt[:],
                    axis=mybir.AxisListType.X,
                    op=ALU.add,
                )
            nc.sync.dma_start(out=xs_ap, in_=xs_t[:])
    nc.compile()
    return nc


# --------------------------------------------------------------------------
# launch 2: the GAT edge math on dense degree-banded grids
# --------------------------------------------------------------------------


@functools.cache
def _build_nc2(w0: int, w1: int, nb1: int):
    comb = _register_comb_op()
    elu_tail = _register_elu_tail_op()
    FB0 = B * w0
    FB1 = nb1 * w1
    PDT = BF16 if USE_BF16_PROD else F32

    nc = bacc.Bacc("TRN2", target_bir_lowering=False, debug=False, num_devices=NCORES)
    # all grid inputs arrive pre-swizzled: [128 partitions, per-partition data]
    v_d = nc.dram_tensor("v", [128, B], F32, kind="ExternalInput")
    pc_d = nc.dram_tensor("pc", [128, B], F32, kind="ExternalInput")
    # br = [cs(8) cd(8) -M(8) exp(-M)(8)]: weight-derived scalars, host-folded
    br_d = nc.dram_tensor("br", [1, 32], F32, kind="ExternalInput")
    ws_d = nc.dram_tensor("wsvec", [1, H * D], F32, kind="ExternalInput")
    b_d = nc.dram_tensor("bvec", [1, H * D], F32, kind="ExternalInput")
    u0_d = nc.dram_tensor("U0", [128, FB0], F32, kind="ExternalInput")
    u016_d = nc.dram_tensor("U0_16", [128, FB0], PDT, kind="ExternalInput")
    if nb1:
        u1_d = nc.dram_tensor("U1", [128, FB1], F32, kind="ExternalInput")
        u116_d = nc.dram_tensor("U1_16", [128, FB1], PDT, kind="ExternalInput")
    out_d = nc.dram_tensor("out", [128, B * H * D], F32, kind="ExternalOutput")

    zin_d = nc.dram_tensor("zin", [1, H], F32)
    zout_d = nc.dram_tensor("zout", [1, H], F32, addr_space="Shared")

    with tile.TileContext(nc) as tc:
        with (
            tc.tile_pool(name="singles", bufs=1) as singles,
            tc.tile_pool(name="work", bufs=2) as work,
            tc.tile_pool(name="small", bufs=2) as small,
            tc.tile_pool(name="psum", bufs=2, space="PSUM") as psum,
        ):
            # ---- small loads first ----
            V1 = singles.tile([128, B], F32)
            nc.sync.dma_start(out=V1[:], in_=v_d.ap())
            PC1 = singles.tile([128, B], F32)
            nc.sync.dma_start(out=PC1[:], in_=pc_d.ap())
            BR = singles.tile([128, 32], F32)
            nc.sync.dma_start(out=BR[:], in_=_ap_with(br_d.ap(), [[0, 128], [1, 32]]))
            ws_row = singles.tile([1, H * D], F32)
            nc.sync.dma_start(out=ws_row[:], in_=ws_d.ap())
            bias_t = singles.tile([1, H * D], F32)
            nc.sync.dma_start(out=bias_t[:], in_=b_d.ap())
            # ---- big grid loads ----
            U0 = singles.tile([128, FB0], F32)
            nc.sync.dma_start(out=U0[:], in_=u0_d.ap())
            if nb1:
                U1 = singles.tile([128, FB1], F32)
                nc.sync.dma_start(out=U1[:], in_=u1_d.ap())
            U016 = singles.tile([128, FB0], PDT)
            nc.sync.dma_start(out=U016[:], in_=u016_d.ap())
            if nb1:
                U116 = singles.tile([128, FB1], PDT)
                nc.sync.dma_start(out=U116[:], in_=u116_d.ap())

            ones_col = singles.tile([128, 1], F32)
            nc.vector.memset(ones_col[:], 1.0)
            ones_row = singles.tile([1, 128], F32)
            nc.vector.memset(ones_row[:], 1.0)

            # ---- V broadcast along occ for both bands ----
            VB0 = singles.tile([128, FB0], F32)
            nc.vector.tensor_copy(
                out=VB0[:], in_=_ap_with(V1[:], [None, [1, B], [0, w0]]),
            )
            if nb1:
                VB1 = singles.tile([128, FB1], F32)
                nc.vector.tensor_copy(
                    out=VB1[:], in_=_ap_with(V1[:], [None, [1, nb1], [0, w1]]),
                )

            # ---- phase 1: exp grids for all heads (Z ready early) ----
            zgrid = singles.tile([128, 2 * H], F32)   # [band0 | band1]
            T = singles.tile([128, H, B], F32)
            p0s = singles.tile([128, H, FB0], PDT, name="p0s")
            p1s = singles.tile([128, H, FB1], PDT, name="p1s") if nb1 else None

            for h in range(H):
                w0_t = work.tile([128, FB0], F32, tag="w0", name=f"w0t{h}")
                nc.vector._custom_dve(
                    comb, out=w0_t[:], in0=U0[:], in1=VB0[:],
                    s0=BR[:, h:h + 1], s1=BR[:, 8 + h:9 + h], imm2=0.8,
                )
                nc.scalar.activation(
                    p0s[:, h, :], w0_t[:], ACTF.Exp,
                    bias=BR[:, 16 + h:17 + h], scale=1.0,
                    accum_out=zgrid[:, h:h + 1],
                )
                if nb1:
                    w1_t = work.tile([128, FB1], F32, tag="w1", name=f"w1t{h}")
                    nc.vector._custom_dve(
                        comb, out=w1_t[:], in0=U1[:], in1=VB1[:],
                        s0=BR[:, h:h + 1], s1=BR[:, 8 + h:9 + h], imm2=0.8,
                    )
                    nc.scalar.activation(
                        p1s[:, h, :], w1_t[:], ACTF.Exp,
                        bias=BR[:, 16 + h:17 + h], scale=1.0,
                        accum_out=zgrid[:, H + h:H + h + 1],
                    )

            # ---- batched pad correction ----
            # padz[:, h] = em[h] * sum_b pc[b] * exp(lrelu(cd[h]*v[b]))
            padz = singles.tile([128, H], F32)
            cdv = small.tile([128, H, B], F32, tag="cdv")
            nc.vector.tensor_tensor(
                out=cdv[:],
                in0=_ap_with(V1[:], [None, [0, H], [1, B]]),
                in1=_ap_with(BR[:], [None, [1, H], [0, B]], offset_elems=8),
                op=ALU.mult,
            )
            cdm = small.tile([128, H, B], F32, tag="cdm")
            nc.vector.tensor_scalar_min(cdm[:], cdv[:], 0.0)
            nc.vector.tensor_scalar(
                out=cdm[:], in0=cdm[:], scalar1=-0.8, scalar2=None, op0=ALU.mult,
            )
            nc.vector.tensor_add(cdm[:], cdm[:], cdv[:])
            pexp = small.tile([128, H, B], F32, tag="pexp")
            nc.scalar.activation(pexp[:], cdm[:], ACTF.Exp)
            pw = small.tile([128, H, B], F32, tag="pw")
            nc.vector.tensor_tensor(
                out=pw[:],
                in0=_ap_with(PC1[:], [None, [0, H], [1, B]]),
                in1=_ap_with(BR[:], [None, [1, H], [0, B]], offset_elems=24),
                op=ALU.mult,
            )
            nc.vector.tensor_mul(pexp[:], pexp[:], pw[:])
            nc.vector.tensor_reduce(
                out=padz[:], in_=pexp[:], axis=mybir.AxisListType.X, op=ALU.add,
            )

            # ---- Z partials out + AllReduce (overlaps phase 2) ----
            zv = singles.tile([128, H], F32)
            if nb1:
                nc.vector.tensor_add(zv[:], zgrid[:, 0:H], zgrid[:, H:2 * H])
                nc.vector.tensor_sub(zv[:], zv[:], padz[:])
            else:
                nc.vector.tensor_sub(zv[:], zgrid[:, 0:H], padz[:])
            psum_z = psum.tile([1, H], F32)
            nc.tensor.matmul(psum_z[:], ones_col[:], zv[:])
            zr = singles.tile([1, H], F32)
            nc.vector.tensor_copy(zr[:], psum_z[:])
            nc.sync.dma_start(out=zin_d.ap(), in_=zr[:])
            nc.gpsimd.collective_compute(
                "AllReduce",
                ALU.add,
                replica_groups=[list(range(NCORES))],
                ins=[zin_d.ap()],
                outs=[zout_d.ap()],
            )

            # ---- phase 2: t = sum_j u * p per block (overlaps the AR) ----
            for h in range(H):
                prod0 = work.tile([128, FB0], PDT, tag="prod0", name=f"pr0t{h}")
                nc.vector.tensor_mul(prod0[:], U016[:], p0s[:, h, :])
                nc.vector.tensor_reduce(
                    out=T[:, h, :],
                    in_=prod0[:].rearrange("p (b j) -> p b j", b=B),
                    axis=mybir.AxisListType.X,
                    op=ALU.add,
                )
                if nb1:
                    prod1 = work.tile([128, FB1], PDT, tag="prod1", name=f"pr1t{h}")
                    nc.vector.tensor_mul(prod1[:], U116[:], p1s[:, h, :])
                    t1 = small.tile([128, nb1], F32, tag="t1", name=f"t1t{h}")
                    nc.vector.tensor_reduce(
                        out=t1[:],
                        in_=prod1[:].rearrange("p (b j) -> p b j", b=nb1),
                        axis=mybir.AxisListType.X,
                        op=ALU.add,
                    )
                    nc.vector.tensor_add(T[:, h, 0:nb1], T[:, h, 0:nb1], t1[:])

            zr2 = singles.tile([1, H], F32)
            nc.sync.dma_start(out=zr2[:], in_=zout_d.ap())
            zinv = singles.tile([1, H], F32)
            nc.vector.reciprocal(zinv[:], zr2[:])

            # wz[h*8+d] = ws[h,d]*zinv[h]; pack [wz(64) bias(64)], bcast via PE
            wzrow = singles.tile([1, 128], F32)
            nc.vector.tensor_tensor(
                out=wzrow[:, 0:64].rearrange("p (h d) -> p h d", h=H),
                in0=ws_row[:].rearrange("p (h d) -> p h d", h=H),
                in1=_ap_with(zinv[:], [None, [1, H], [0, D]]),
                op=ALU.mult,
            )
            nc.vector.tensor_copy(wzrow[:, 64:128], bias_t[:])
            psum_wb = psum.tile([128, 128], F32)
            nc.tensor.matmul(psum_wb[:], ones_row[:], wzrow[:])
            WB = singles.tile([128, 128], F32)
            nc.vector.tensor_copy(WB[:], psum_wb[:])

            # ---- epilogue: out = elu(t*wz + bias), 7 chunks of 14 blocks ----
            # engine split: DVE (y-mult, max-add), ACT (relu(-s), exp(-r)),
            # GPSIMD (bias-add, final add)
            CB = 14
            t_pstep = T[:].ap[0][0]
            wb_pstep = WB[:].ap[0][0]
            out_ap = out_d.ap().rearrange("p (b c) -> p b c", c=H * D)
            with (
                tc.tile_pool(name="epi", bufs=2) as epi,
            ):
                for ch in range(B // CB):
                    y_t = epi.tile([128, CB, H * D], F32, tag="y")
                    # y[p, b, h*8+d] = T[p, h, ch*CB+b] * WB[p, h*8+d]
                    nc.vector.tensor_tensor(
                        out=y_t[:],
                        in0=_ap_with(T[:], [[t_pstep, 128], [1, CB], [B, H], [0, D]],
                                     offset_elems=ch * CB),
                        in1=_ap_with(WB[:], [[wb_pstep, 128], [0, CB], [1, H * D]]),
                        op=ALU.mult,
                    )
                    s_t = epi.tile([128, CB, H * D], F32, tag="s")
                    eng = nc.gpsimd if ch % 2 == 0 else nc.vector
                    eng.tensor_tensor(
                        out=s_t[:],
                        in0=y_t[:],
                        in1=_ap_with(WB[:], [[wb_pstep, 128], [0, CB], [1, H * D]],
                                     offset_elems=64),
                        op=ALU.add,
                    )
                    # r = relu(-s) = -min(s,0);  q = exp(-r) = exp(min(s,0))
                    r_t = epi.tile([128, CB, H * D], F32, tag="r")
                    nc.scalar.activation(r_t[:], s_t[:], ACTF.Relu, scale=-1.0)
                    q_t = epi.tile([128, CB, H * D], F32, tag="q")
                    nc.scalar.activation(q_t[:], r_t[:], ACTF.Exp, scale=-1.0)
                    o_t = epi.tile([128, CB, H * D], F32, tag="o")
                    # elu(s) = max(s,0) + exp(min(s,0)) - 1
                    nc.vector._custom_dve(
                        elu_tail, out=o_t[:].rearrange("p b c -> p (b c)"),
                        in0=s_t[:].rearrange("p b c -> p (b c)"),
                        in1=q_t[:].rearrange("p b c -> p (b c)"),
                    )
                    nc.sync.dma_start(
                        out=out_ap[:, ch * CB:(ch + 1) * CB, :], in_=o_t[:],
                    )
    nc.compile()
    return nc


# --------------------------------------------------------------------------
# host orchestration
# --------------------------------------------------------------------------


def _swz(arr):
    """[PN, ...] row-major (block-major node ids: ln = b*128 + p) ->
    device layout [128, B, ...] flattened per partition."""
    rest = arr.shape[1:]
    return np.ascontiguousarray(
        arr.reshape(B, 128, *rest).transpose(1, 0, *range(2, 2 + len(rest)))
    ).reshape(128, -1)


def kernel(x, edge_index, W, a, bias):
    x = np.ascontiguousarray(np.asarray(x), dtype=np.float32)
    ei = np.asarray(edge_index)
    src = ei[0].astype(np.int64)
    dst = ei[1].astype(np.int64)
    W = np.asarray(W, dtype=np.float32)
    a = np.asarray(a, dtype=np.float32)
    bias = np.asarray(bias, dtype=np.float32)

    cores = list(range(NCORES))

    # ---- launch 1: xsum ----
    nc1 = _build_nc1()
    in1 = []
    for c in cores:
        xp = np.zeros((PN, F_IN), dtype=np.float32)
        xp[:NPC] = x[c * NPC:(c + 1) * NPC]
        in1.append({"x": xp})
    r1 = run_bass_kernel_spmd(nc1, in1, cores, trace=_TRACE)
    xsum = np.concatenate([r1.results[c]["xsum"][:NPC] for c in cores])
    LAST_STATS["launch1_ns"] = r1.exec_time_ns

    gmax = float(np.abs(xsum).max())

    # ---- host: degree-sorted banded grids (index work + one gather) ----
    deg = np.bincount(dst, minlength=N)
    degc = deg.reshape(NCORES, NPC)
    dmax = int(deg.max())

    # pick band split minimizing total slots
    best = None
    for cand in range(8, dmax + 1):
        cnt = int((degc > cand).sum(1).max())
        nb = -(-cnt // 128) if cnt else 0
        slots = PN * cand + nb * 128 * (dmax - cand)
        if best is None or slots < best[0]:
            best = (slots, cand, nb)
    _, W0, nb1 = best
    W1 = dmax - W0 if nb1 else 0

    # per-core degree-descending node order
    deg_pad = np.zeros((NCORES, PN), dtype=np.int64)
    deg_pad[:, :NPC] = degc
    perm = np.argsort(-deg_pad, axis=1, kind="stable")       # [NC, PN]
    inv_perm = np.argsort(perm, axis=1)                      # orig ln -> sorted row

    # per-edge target slots
    order = np.argsort(dst, kind="stable")
    sdst = dst[order]
    usrc = xsum[src[order]]
    starts = np.concatenate([[0], np.cumsum(deg)[:-1]])
    occ = np.arange(E, dtype=np.int64) - starts[sdst]
    core_idx = sdst // NPC
    row = inv_perm[core_idx, sdst % NPC]                     # sorted row id

    U0_all = np.zeros((NCORES, PN, W0), dtype=np.float32)
    m0 = occ < W0
    U0_all[core_idx[m0], row[m0], occ[m0]] = usrc[m0]
    if nb1:
        U1_all = np.zeros((NCORES, nb1 * 128, W1), dtype=np.float32)
        m1 = ~m0
        U1_all[core_idx[m1], row[m1], occ[m1] - W0] = usrc[m1]

    cap = np.full(PN, W0, dtype=np.float32)
    cap[:nb1 * 128] += W1
    sdeg = np.take_along_axis(deg_pad, perm, axis=1).astype(np.float32)
    pc_all = cap[None, :] - sdeg                             # [NC, PN]

    v_all = np.zeros((NCORES, PN), dtype=np.float32)
    v_all[:, :NPC] = xsum.reshape(NCORES, NPC)
    v_all = np.take_along_axis(v_all, perm, axis=1)

    pdt = ml_dtypes.bfloat16 if USE_BF16_PROD else np.float32

    # weight-derived scalars (pure weight preprocessing)
    wsum = W.sum(1)                                 # [H, D]
    cs = (wsum * a[:, :D, 0]).sum(1)                # [H]
    cd = (wsum * a[:, D:, 0]).sum(1)
    M = (np.abs(cs) + np.abs(cd)) * gmax - 40.0
    brvec = np.concatenate([cs, cd, -M, np.exp(-M)]).astype(np.float32)[None, :]
    wsvec = np.ascontiguousarray(wsum.reshape(1, H * D)).astype(np.float32)
    bvec = np.ascontiguousarray(bias.reshape(1, H * D))

    # ---- launch 2 ----
    nc2 = _build_nc2(W0, W1, nb1)
    in2 = []
    for c in cores:
        m = {
            "v": _swz(v_all[c][:, None]),
            "pc": _swz(pc_all[c][:, None]),
            "br": brvec,
            "wsvec": wsvec,
            "bvec": bvec,
            "U0": _swz(U0_all[c]),
        }
        m["U0_16"] = m["U0"].astype(pdt)
        if nb1:
            u1 = np.ascontiguousarray(
                U1_all[c].reshape(nb1, 128, W1).transpose(1, 0, 2)
            ).reshape(128, -1)
            m["U1"] = u1
            m["U1_16"] = u1.astype(pdt)
        in2.append(m)
    r2 = run_bass_kernel_spmd(nc2, in2, cores, trace=_TRACE)
    LAST_STATS["launch2_ns"] = r2.exec_time_ns

    out = np.empty((N, H * D), dtype=np.float32)
    for c in cores:
        o = r2.results[c]["out"].reshape(128, B, H * D).transpose(1, 0, 2)
        o = o.reshape(PN, H * D)          # rows in sorted order
        sel = perm[c] < NPC
        out[c * NPC + perm[c][sel]] = o[sel]
    return out


# revision 13
# speedup vs baseline: 1.0488x; 1.0030x over previous
"""GAT layer (global-softmax variant) as a 2-launch Bass kernel on 8 trn2 cores.

Math (the reference einsum 'hid,nf->hnd' has no shared index, so it factors):
    xsum[n]  = sum_f x[n, f]
    wsum[h,d]= sum_i W[h, i, d]            -> Wh[h,n,d] = wsum[h,d] * xsum[n]
    cs[h]    = sum_d wsum[h,d] * a[h, d, 0]
    cd[h]    = sum_d wsum[h,d] * a[h, D+d, 0]
    e[h,k]   = leakyrelu(cs[h]*xsum[src_k] + cd[h]*xsum[dst_k], 0.2)
    alpha    = softmax over all E edges (per head)
    t[h,n]   = sum_{k: dst_k = n} xsum[src_k] * alpha[h,k]
    out[n, h*D+d] = elu(wsum[h,d] * t[h,n] + bias[h,d])

Sharding: nodes (and their incoming edges) are sharded across 8 cores; the
host additionally sorts each core's nodes by in-degree (descending) and
builds two dense degree-padded grids of gathered xsum[src] values: a base
band of width W0 covering every node and a narrow spill band covering only
the first nb1 blocks (the high-degree nodes).  The device then runs only
dense passes: a fused DVE op computes leakyrelu(cs*U + cd*V), ACT
exponentiates (with a per-head shift M; softmax is shift-invariant), a
bf16 multiply + block-reduce yields t, and an 8-float AllReduce combines
the softmax denominators.  Pad slots hold u=0 so they add 0 to t; their
exp(lrelu(cd*v)-M) contribution to Z is removed analytically via the
per-node pad-count correction.  All tensors cross PCIe pre-swizzled into
the device's [partition, block] layout so every DMA is contiguous.
"""

import os
import sys

for _p in ("/opt/trn_rl_repo", "/root/.axon_site/_ro/trn_rl_repo"):
    if os.path.isdir(_p) and _p not in sys.path:
        sys.path.insert(0, _p)

import dataclasses
import functools

import numpy as np
import ml_dtypes

import concourse.bass as bass
import concourse.tile as tile
from concourse import bacc, mybir
from concourse.bass_utils import run_bass_kernel_spmd

F32 = mybir.dt.float32
BF16 = mybir.dt.bfloat16
ALU = mybir.AluOpType
ACTF = mybir.ActivationFunctionType

NCORES = 8
N = 100000
E = 1600000
H = 8
D = 8
F_IN = 128
NPC = N // NCORES          # 12500 real nodes per core
B = 98                     # blocks per partition (128 * 98 = 12544)
PN = 128 * B               # padded nodes per core (12544)

# bf16 for the u*p multiply + block reduce (t only; Z stays f32)
USE_BF16_PROD = bool(int(os.environ.get("GAT_BF16", "1")))

# Populated by kernel() for test harnesses to inspect.
LAST_STATS = {}

_TRACE = bool(int(os.environ.get("GAT_TRACE", "0")))

# --------------------------------------------------------------------------
# custom DVE op: out = leakyrelu(in0*s0 + in1*s1) with slope (1 - imm2)
#   z = Src0*C0 + Src1*C1 ; out = z - min(z, 0)*C2      (imm2 = 0.8 -> 0.2)
# --------------------------------------------------------------------------


def _comb_ref(in0, in1, s0, s1, imm2):
    z = in0.astype(np.float32) * s0 + in1.astype(np.float32) * s1
    return (z - np.minimum(z, 0.0) * imm2).astype(np.float32)


@functools.cache
def _register_comb_op():
    import concourse.dve_ops as dve_ops
    from concourse.dve_spec import Spec, Src0, Src1, C0, C1, C2, Zero, minn, lower
    from concourse.dve_uop import DveOpSpec

    name = "GAT_COMB_LRELU"
    if name in dve_ops._SUB_OPCODE_FOR_NAME:
        return next(op for op in dve_ops.OPS if op.name == name)

    z = Src0 * C0 + Src1 * C1
    spec = Spec(body=z - minn(z, Zero) * C2, reference=_comb_ref)

    row = dve_ops._CUSTOM_DVE_ROW_BASE + len(dve_ops.OPS)
    assert row < 0x20
    shas = {}
    for ver in ("v3", "v4"):
        s = DveOpSpec(name=name, opcode=row, uops=lower(spec, ver=ver), rd1_en=True)
        shas[ver] = s.sha(ver)
    op = dve_ops.DveOp(name, spec, subdim=False, uops_sha=shas)
    dve_ops.OPS.append(op)
    dve_ops.CUSTOM_DVE_SPECS[name] = spec
    dve_ops._SUB_OPCODE_FOR_NAME[name] = row
    return op


def _elu_tail_ref(in0, in1, s0, s1, imm2):
    return (np.maximum(in0.astype(np.float32), 0.0) + in1 - 1.0).astype(np.float32)


@functools.cache
def _register_elu_tail_op():
    import concourse.dve_ops as dve_ops
    from concourse.dve_spec import Spec, Src0, Src1, Zero, One, maxx, lower
    from concourse.dve_uop import DveOpSpec

    name = "GAT_ELU_TAIL"
    if name in dve_ops._SUB_OPCODE_FOR_NAME:
        return next(op for op in dve_ops.OPS if op.name == name)

    spec = Spec(body=maxx(Src0, Zero) + Src1 - One, reference=_elu_tail_ref)
    row = dve_ops._CUSTOM_DVE_ROW_BASE + len(dve_ops.OPS)
    assert row < 0x20
    shas = {}
    for ver in ("v3", "v4"):
        sp = DveOpSpec(name=name, opcode=row, uops=lower(spec, ver=ver), rd1_en=True)
        shas[ver] = sp.sha(ver)
    op = dve_ops.DveOp(name, spec, subdim=False, uops_sha=shas)
    dve_ops.OPS.append(op)
    dve_ops.CUSTOM_DVE_SPECS[name] = spec
    dve_ops._SUB_OPCODE_FOR_NAME[name] = row
    return op


def _ap_with(ap: bass.AP, dims, offset_elems=0):
    """Clone `ap` with explicit [step, count] dims (element units).

    dims[0] is the partition dim; pass None to keep the AP's own.
    """
    dims = [list(ap.ap[0]) if d is None else list(d) for d in dims]
    return dataclasses.replace(ap, ap=dims, offset=ap.offset + offset_elems)


# --------------------------------------------------------------------------
# launch 1: xsum over features (per core: x slice [PN, 128] -> xsum [PN])
# --------------------------------------------------------------------------


@functools.cache
def _build_nc1():
    nc = bacc.Bacc("TRN2", target_bir_lowering=False, debug=False)
    x_d = nc.dram_tensor("x", [PN, F_IN], F32, kind="ExternalInput")
    xs_d = nc.dram_tensor("xsum", [PN], F32, kind="ExternalOutput")

    x_ap = x_d.ap().rearrange("(p b) f -> p b f", p=128)       # [128, 98, 128]
    xs_ap = xs_d.ap().rearrange("(p b) -> p b", p=128)         # [128, 98]

    CH = 14
    BPC = B // CH
    with tile.TileContext(nc) as tc:
        with (
            tc.tile_pool(name="xin", bufs=3) as xin_pool,
            tc.tile_pool(name="xs", bufs=1) as xs_pool,
        ):
            xs_t = xs_pool.tile([128, B], F32)
            for ch in range(CH):
                xt = xin_pool.tile([128, BPC, F_IN], F32)
                nc.sync.dma_start(out=xt[:], in_=x_ap[:, ch * BPC:(ch + 1) * BPC, :])
                nc.vector.tensor_reduce(
                    out=xs_t[:, ch * BPC:(ch + 1) * BPC],
                    in_=xt[:],
                    axis=mybir.AxisListType.X,
                    op=ALU.add,
                )
            nc.sync.dma_start(out=xs_ap, in_=xs_t[:])
    nc.compile()
    return nc


# --------------------------------------------------------------------------
# launch 2: the GAT edge math on dense degree-banded grids
# --------------------------------------------------------------------------


@functools.cache
def _build_nc2(w0: int, w1: int, nb1: int):
    comb = _register_comb_op()
    elu_tail = _register_elu_tail_op()
    FB0 = B * w0
    FB1 = nb1 * w1
    PDT = BF16 if USE_BF16_PROD else F32

    nc = bacc.Bacc("TRN2", target_bir_lowering=False, debug=False, num_devices=NCORES)
    # all grid inputs arrive pre-swizzled: [128 partitions, per-partition data]
    v_d = nc.dram_tensor("v", [128, B], F32, kind="ExternalInput")
    pc_d = nc.dram_tensor("pc", [128, B], F32, kind="ExternalInput")
    # br = [cs(8) cd(8) -M(8) exp(-M)(8)]: weight-derived scalars, host-folded
    br_d = nc.dram_tensor("br", [1, 32], F32, kind="ExternalInput")
    ws_d = nc.dram_tensor("wsvec", [1, H * D], F32, kind="ExternalInput")
    b_d = nc.dram_tensor("bvec", [1, H * D], F32, kind="ExternalInput")
    u0_d = nc.dram_tensor("U0", [128, FB0], F32, kind="ExternalInput")
    u016_d = nc.dram_tensor("U0_16", [128, FB0], PDT, kind="ExternalInput")
    if nb1:
        u1_d = nc.dram_tensor("U1", [128, FB1], F32, kind="ExternalInput")
        u116_d = nc.dram_tensor("U1_16", [128, FB1], PDT, kind="ExternalInput")
    out_d = nc.dram_tensor("out", [128, B * H * D], F32, kind="ExternalOutput")

    zin_d = nc.dram_tensor("zin", [1, H], F32)
    zout_d = nc.dram_tensor("zout", [NCORES, H], F32, addr_space="Shared")

    with tile.TileContext(nc) as tc:
        with (
            tc.tile_pool(name="singles", bufs=1) as singles,
            tc.tile_pool(name="work", bufs=2) as work,
            tc.tile_pool(name="small", bufs=2) as small,
            tc.tile_pool(name="psum", bufs=2, space="PSUM") as psum,
        ):
            # ---- small loads first ----
            V1 = singles.tile([128, B], F32)
            nc.sync.dma_start(out=V1[:], in_=v_d.ap())
            PC1 = singles.tile([128, B], F32)
            nc.sync.dma_start(out=PC1[:], in_=pc_d.ap())
            BR = singles.tile([128, 32], F32)
            nc.sync.dma_start(out=BR[:], in_=_ap_with(br_d.ap(), [[0, 128], [1, 32]]))
            ws_row = singles.tile([1, H * D], F32)
            nc.sync.dma_start(out=ws_row[:], in_=ws_d.ap())
            bias_t = singles.tile([1, H * D], F32)
            nc.sync.dma_start(out=bias_t[:], in_=b_d.ap())
            # ---- big grid loads ----
            U0 = singles.tile([128, FB0], F32)
            nc.sync.dma_start(out=U0[:], in_=u0_d.ap())
            if nb1:
                U1 = singles.tile([128, FB1], F32)
                nc.sync.dma_start(out=U1[:], in_=u1_d.ap())
            U016 = singles.tile([128, FB0], PDT)
            nc.sync.dma_start(out=U016[:], in_=u016_d.ap())
            if nb1:
                U116 = singles.tile([128, FB1], PDT)
                nc.sync.dma_start(out=U116[:], in_=u116_d.ap())

            ones_col = singles.tile([128, 1], F32)
            nc.vector.memset(ones_col[:], 1.0)
            ones_row = singles.tile([1, 128], F32)
            nc.vector.memset(ones_row[:], 1.0)

            # ---- V broadcast along occ for both bands ----
            VB0 = singles.tile([128, FB0], F32)
            nc.gpsimd.tensor_copy(
                out=VB0[:], in_=_ap_with(V1[:], [None, [1, B], [0, w0]]),
            )
            if nb1:
                VB1 = singles.tile([128, FB1], F32)
                nc.gpsimd.tensor_copy(
                    out=VB1[:], in_=_ap_with(V1[:], [None, [1, nb1], [0, w1]]),
                )

            # ---- phase 1: exp grids for all heads (Z ready early) ----
            zgrid = singles.tile([128, 2 * H], F32)   # [band0 | band1]
            T = singles.tile([128, H, B], F32)
            p0s = singles.tile([128, H, FB0], PDT, name="p0s")
            p1s = singles.tile([128, H, FB1], PDT, name="p1s") if nb1 else None

            for h in range(H):
                w0_t = work.tile([128, FB0], F32, tag="w0", name=f"w0t{h}")
                nc.vector._custom_dve(
                    comb, out=w0_t[:], in0=U0[:], in1=VB0[:],
                    s0=BR[:, h:h + 1], s1=BR[:, 8 + h:9 + h], imm2=0.8,
                )
                nc.scalar.activation(
                    p0s[:, h, :], w0_t[:], ACTF.Exp,
                    bias=BR[:, 16 + h:17 + h], scale=1.0,
                    accum_out=zgrid[:, h:h + 1],
                )
                if nb1:
                    w1_t = work.tile([128, FB1], F32, tag="w1", name=f"w1t{h}")
                    nc.vector._custom_dve(
                        comb, out=w1_t[:], in0=U1[:], in1=VB1[:],
                        s0=BR[:, h:h + 1], s1=BR[:, 8 + h:9 + h], imm2=0.8,
                    )
                    nc.scalar.activation(
                        p1s[:, h, :], w1_t[:], ACTF.Exp,
                        bias=BR[:, 16 + h:17 + h], scale=1.0,
                        accum_out=zgrid[:, H + h:H + h + 1],
                    )

            # ---- batched pad correction ----
            # padz[:, h] = em[h] * sum_b pc[b] * exp(lrelu(cd[h]*v[b]))
            padz = singles.tile([128, H], F32)
            cdv = small.tile([128, H, B], F32, tag="cdv")
            nc.vector.tensor_tensor(
                out=cdv[:],
                in0=_ap_with(V1[:], [None, [0, H], [1, B]]),
                in1=_ap_with(BR[:], [None, [1, H], [0, B]], offset_elems=8),
                op=ALU.mult,
            )
            cdm = small.tile([128, H, B], F32, tag="cdm")
            nc.vector.tensor_scalar_min(cdm[:], cdv[:], 0.0)
            nc.vector.tensor_scalar(
                out=cdm[:], in0=cdm[:], scalar1=-0.8, scalar2=None, op0=ALU.mult,
            )
            nc.vector.tensor_add(cdm[:], cdm[:], cdv[:])
            pexp = small.tile([128, H, B], F32, tag="pexp")
            nc.scalar.activation(pexp[:], cdm[:], ACTF.Exp)
            pw = small.tile([128, H, B], F32, tag="pw")
            nc.vector.tensor_tensor(
                out=pw[:],
                in0=_ap_with(PC1[:], [None, [0, H], [1, B]]),
                in1=_ap_with(BR[:], [None, [1, H], [0, B]], offset_elems=24),
                op=ALU.mult,
            )
            nc.vector.tensor_mul(pexp[:], pexp[:], pw[:])
            nc.vector.tensor_reduce(
                out=padz[:], in_=pexp[:], axis=mybir.AxisListType.X, op=ALU.add,
            )

            # ---- Z partials out + AllReduce (overlaps phase 2) ----
            zv = singles.tile([128, H], F32)
            if nb1:
                nc.vector.tensor_add(zv[:], zgrid[:, 0:H], zgrid[:, H:2 * H])
                nc.vector.tensor_sub(zv[:], zv[:], padz[:])
            else:
                nc.vector.tensor_sub(zv[:], zgrid[:, 0:H], padz[:])
            psum_z = psum.tile([1, H], F32)
            nc.tensor.matmul(psum_z[:], ones_col[:], zv[:])
            zr = singles.tile([1, H], F32)
            nc.vector.tensor_copy(zr[:], psum_z[:])
            nc.sync.dma_start(out=zin_d.ap(), in_=zr[:])
            nc.gpsimd.collective_compute(
                "AllGather",
                ALU.bypass,
                replica_groups=[list(range(NCORES))],
                ins=[zin_d.ap()],
                outs=[zout_d.ap()],
            )

            # ---- phase 2: t = sum_j u * p per block (overlaps the AR) ----
            for h in range(H):
                prod0 = work.tile([128, FB0], PDT, tag="prod0", name=f"pr0t{h}")
                meng = nc.gpsimd if h in (3, 6) else nc.vector
                meng.tensor_mul(prod0[:], U016[:], p0s[:, h, :])
                nc.vector.tensor_reduce(
                    out=T[:, h, :],
                    in_=prod0[:].rearrange("p (b j) -> p b j", b=B),
                    axis=mybir.AxisListType.X,
                    op=ALU.add,
                )
                if nb1:
                    prod1 = work.tile([128, FB1], PDT, tag="prod1", name=f"pr1t{h}")
                    nc.vector.tensor_mul(prod1[:], U116[:], p1s[:, h, :])
                    t1 = small.tile([128, nb1], F32, tag="t1", name=f"t1t{h}")
                    nc.vector.tensor_reduce(
                        out=t1[:],
                        in_=prod1[:].rearrange("p (b j) -> p b j", b=nb1),
                        axis=mybir.AxisListType.X,
                        op=ALU.add,
                    )
                    nc.vector.tensor_add(T[:, h, 0:nb1], T[:, h, 0:nb1], t1[:])

            zgat = singles.tile([NCORES, H], F32)
            nc.sync.dma_start(out=zgat[:], in_=zout_d.ap())
            psum_zt = psum.tile([1, H], F32, name="psum_zt")
            nc.tensor.matmul(psum_zt[:], ones_col[0:NCORES, :], zgat[:])
            zr2 = singles.tile([1, H], F32)
            nc.vector.tensor_copy(zr2[:], psum_zt[:])
            zinv = singles.tile([1, H], F32)
            nc.vector.reciprocal(zinv[:], zr2[:])

            # wz[h*8+d] = ws[h,d]*zinv[h]; pack [wz(64) bias(64)], bcast via PE
            wzrow = singles.tile([1, 128], F32)
            nc.vector.tensor_tensor(
                out=wzrow[:, 0:64].rearrange("p (h d) -> p h d", h=H),
                in0=ws_row[:].rearrange("p (h d) -> p h d", h=H),
                in1=_ap_with(zinv[:], [None, [1, H], [0, D]]),
                op=ALU.mult,
            )
            nc.vector.tensor_copy(wzrow[:, 64:128], bias_t[:])
            psum_wb = psum.tile([128, 128], F32)
            nc.tensor.matmul(psum_wb[:], ones_row[:], wzrow[:])
            WB = singles.tile([128, 128], F32)
            nc.vector.tensor_copy(WB[:], psum_wb[:])

            # ---- epilogue: out = elu(t*wz + bias), 7 chunks of 14 blocks ----
            # engine split: DVE (y-mult, max-add), ACT (relu(-s), exp(-r)),
            # GPSIMD (bias-add, final add)
            CB = 14
            t_pstep = T[:].ap[0][0]
            wb_pstep = WB[:].ap[0][0]
            out_ap = out_d.ap().rearrange("p (b c) -> p b c", c=H * D)
            with (
                tc.tile_pool(name="epi", bufs=2) as epi,
            ):
                for ch in range(B // CB):
                    y_t = epi.tile([128, CB, H * D], F32, tag="y")
                    # y[p, b, h*8+d] = T[p, h, ch*CB+b] * WB[p, h*8+d]
                    nc.vector.tensor_tensor(
                        out=y_t[:],
                        in0=_ap_with(T[:], [[t_pstep, 128], [1, CB], [B, H], [0, D]],
                                     offset_elems=ch * CB),
                        in1=_ap_with(WB[:], [[wb_pstep, 128], [0, CB], [1, H * D]]),
                        op=ALU.mult,
                    )
                    s_t = epi.tile([128, CB, H * D], F32, tag="s")
                    eng = nc.gpsimd if ch % 2 == 0 else nc.vector
                    eng.tensor_tensor(
                        out=s_t[:],
                        in0=y_t[:],
                        in1=_ap_with(WB[:], [[wb_pstep, 128], [0, CB], [1, H * D]],
                                     offset_elems=64),
                        op=ALU.add,
                    )
                    # r = relu(-s) = -min(s,0);  q = exp(-r) = exp(min(s,0))
                    r_t = epi.tile([128, CB, H * D], F32, tag="r")
                    nc.scalar.activation(r_t[:], s_t[:], ACTF.Relu, scale=-1.0)
                    q_t = epi.tile([128, CB, H * D], F32, tag="q")
                    nc.scalar.activation(q_t[:], r_t[:], ACTF.Exp, scale=-1.0)
                    o_t = epi.tile([128, CB, H * D], F32, tag="o")
                    # elu(s) = max(s,0) + exp(min(s,0)) - 1
                    nc.vector._custom_dve(
                        elu_tail, out=o_t[:].rearrange("p b c -> p (b c)"),
                        in0=s_t[:].rearrange("p b c -> p (b c)"),
                        in1=q_t[:].rearrange("p b c -> p (b c)"),
                    )
                    nc.sync.dma_start(
                        out=out_ap[:, ch * CB:(ch + 1) * CB, :], in_=o_t[:],
                    )
    nc.compile()
    return nc


# --------------------------------------------------------------------------
# host orchestration
# --------------------------------------------------------------------------


def _swz(arr):
    """[PN, ...] row-major (block-major node ids: ln = b*128 + p) ->
    device layout [128, B, ...] flattened per partition."""
    rest = arr.shape[1:]
    return np.ascontiguousarray(
        arr.reshape(B, 128, *rest).transpose(1, 0, *range(2, 2 + len(rest)))
    ).reshape(128, -1)


def kernel(x, edge_index, W, a, bias):
    x = np.ascontiguousarray(np.asarray(x), dtype=np.float32)
    ei = np.asarray(edge_index)
    src = ei[0].astype(np.int64)
    dst = ei[1].astype(np.int64)
    W = np.asarray(W, dtype=np.float32)
    a = np.asarray(a, dtype=np.float32)
    bias = np.asarray(bias, dtype=np.float32)

    cores = list(range(NCORES))

    # ---- launch 1: xsum ----
    nc1 = _build_nc1()
    in1 = []
    for c in cores:
        xp = np.zeros((PN, F_IN), dtype=np.float32)
        xp[:NPC] = x[c * NPC:(c + 1) * NPC]
        in1.append({"x": xp})
    r1 = run_bass_kernel_spmd(nc1, in1, cores, trace=_TRACE)
    xsum = np.concatenate([r1.results[c]["xsum"][:NPC] for c in cores])
    LAST_STATS["launch1_ns"] = r1.exec_time_ns

    gmax = float(np.abs(xsum).max())

    # ---- host: degree-sorted banded grids (index work + one gather) ----
    deg = np.bincount(dst, minlength=N)
    degc = deg.reshape(NCORES, NPC)
    dmax = int(deg.max())

    # pick band split minimizing total slots
    best = None
    for cand in range(8, dmax + 1):
        cnt = int((degc > cand).sum(1).max())
        nb = -(-cnt // 128) if cnt else 0
        slots = PN * cand + nb * 128 * (dmax - cand)
        if best is None or slots < best[0]:
            best = (slots, cand, nb)
    _, W0, nb1 = best
    W1 = dmax - W0 if nb1 else 0

    # per-core degree-descending node order
    deg_pad = np.zeros((NCORES, PN), dtype=np.int64)
    deg_pad[:, :NPC] = degc
    perm = np.argsort(-deg_pad, axis=1, kind="stable")       # [NC, PN]
    inv_perm = np.argsort(perm, axis=1)                      # orig ln -> sorted row

    # per-edge target slots
    order = np.argsort(dst, kind="stable")
    sdst = dst[order]
    usrc = xsum[src[order]]
    starts = np.concatenate([[0], np.cumsum(deg)[:-1]])
    occ = np.arange(E, dtype=np.int64) - starts[sdst]
    core_idx = sdst // NPC
    row = inv_perm[core_idx, sdst % NPC]                     # sorted row id

    U0_all = np.zeros((NCORES, PN, W0), dtype=np.float32)
    m0 = occ < W0
    U0_all[core_idx[m0], row[m0], occ[m0]] = usrc[m0]
    if nb1:
        U1_all = np.zeros((NCORES, nb1 * 128, W1), dtype=np.float32)
        m1 = ~m0
        U1_all[core_idx[m1], row[m1], occ[m1] - W0] = usrc[m1]

    cap = np.full(PN, W0, dtype=np.float32)
    cap[:nb1 * 128] += W1
    sdeg = np.take_along_axis(deg_pad, perm, axis=1).astype(np.float32)
    pc_all = cap[None, :] - sdeg                             # [NC, PN]

    v_all = np.zeros((NCORES, PN), dtype=np.float32)
    v_all[:, :NPC] = xsum.reshape(NCORES, NPC)
    v_all = np.take_along_axis(v_all, perm, axis=1)

    pdt = ml_dtypes.bfloat16 if USE_BF16_PROD else np.float32

    # weight-derived scalars (pure weight preprocessing)
    wsum = W.sum(1)                                 # [H, D]
    cs = (wsum * a[:, :D, 0]).sum(1)                # [H]
    cd = (wsum * a[:, D:, 0]).sum(1)
    M = (np.abs(cs) + np.abs(cd)) * gmax - 40.0
    brvec = np.concatenate([cs, cd, -M, np.exp(-M)]).astype(np.float32)[None, :]
    wsvec = np.ascontiguousarray(wsum.reshape(1, H * D)).astype(np.float32)
    bvec = np.ascontiguousarray(bias.reshape(1, H * D))

    # ---- launch 2 ----
    nc2 = _build_nc2(W0, W1, nb1)
    in2 = []
    for c in cores:
        m = {
            "v": _swz(v_all[c][:, None]),
            "pc": _swz(pc_all[c][:, None]),
            "br": brvec,
            "wsvec": wsvec,
            "bvec": bvec,
            "U0": _swz(U0_all[c]),
        }
        m["U0_16"] = m["U0"].astype(pdt)
        if nb1:
            u1 = np.ascontiguousarray(
                U1_all[c].reshape(nb1, 128, W1).transpose(1, 0, 2)
            ).reshape(128, -1)
            m["U1"] = u1
            m["U1_16"] = u1.astype(pdt)
        in2.append(m)
    r2 = run_bass_kernel_spmd(nc2, in2, cores, trace=_TRACE)
    LAST_STATS["launch2_ns"] = r2.exec_time_ns

    out = np.empty((N, H * D), dtype=np.float32)
    for c in cores:
        o = r2.results[c]["out"].reshape(128, B, H * D).transpose(1, 0, 2)
        o = o.reshape(PN, H * D)          # rows in sorted order
        sel = perm[c] < NPC
        out[c * NPC + perm[c][sel]] = o[sel]
    return out


# revision 14
# speedup vs baseline: 1.1340x; 1.0812x over previous
"""GAT layer (global-softmax variant) as a 2-launch Bass kernel on 8 trn2 cores.

Math (the reference einsum 'hid,nf->hnd' has no shared index, so it factors):
    xsum[n]  = sum_f x[n, f]
    wsum[h,d]= sum_i W[h, i, d]            -> Wh[h,n,d] = wsum[h,d] * xsum[n]
    cs[h]    = sum_d wsum[h,d] * a[h, d, 0]
    cd[h]    = sum_d wsum[h,d] * a[h, D+d, 0]
    e[h,k]   = leakyrelu(cs[h]*xsum[src_k] + cd[h]*xsum[dst_k], 0.2)
    alpha    = softmax over all E edges (per head)
    t[h,n]   = sum_{k: dst_k = n} xsum[src_k] * alpha[h,k]
    out[n, h*D+d] = elu(wsum[h,d] * t[h,n] + bias[h,d])

Sharding: nodes (and their incoming edges) are sharded across 8 cores; the
host additionally sorts each core's nodes by in-degree (descending) and
builds two dense degree-padded grids of gathered xsum[src] values: a base
band of width W0 covering every node and a narrow spill band covering only
the first nb1 blocks (the high-degree nodes).  The device then runs only
dense passes: a fused DVE op computes leakyrelu(cs*U + cd*V), ACT
exponentiates (with a per-head shift M; softmax is shift-invariant), a
bf16 multiply + block-reduce yields t, and an 8-float AllReduce combines
the softmax denominators.  Pad slots hold u=0 so they add 0 to t; their
exp(lrelu(cd*v)-M) contribution to Z is removed analytically via the
per-node pad-count correction.  All tensors cross PCIe pre-swizzled into
the device's [partition, block] layout so every DMA is contiguous.
"""

import os
import sys

for _p in ("/opt/trn_rl_repo", "/root/.axon_site/_ro/trn_rl_repo"):
    if os.path.isdir(_p) and _p not in sys.path:
        sys.path.insert(0, _p)

import dataclasses
import functools

import numpy as np
import ml_dtypes

import concourse.bass as bass
import concourse.tile as tile
from concourse import bacc, mybir
from concourse.bass_utils import run_bass_kernel_spmd

F32 = mybir.dt.float32
BF16 = mybir.dt.bfloat16
ALU = mybir.AluOpType
ACTF = mybir.ActivationFunctionType

NCORES = 8
N = 100000
E = 1600000
H = 8
D = 8
F_IN = 128
NPC = N // NCORES          # 12500 real nodes per core
B = 98                     # blocks per partition (128 * 98 = 12544)
PN = 128 * B               # padded nodes per core (12544)

# bf16 for the u*p multiply + block reduce (t only; Z stays f32)
USE_BF16_PROD = bool(int(os.environ.get("GAT_BF16", "1")))

# Populated by kernel() for test harnesses to inspect.
LAST_STATS = {}

_TRACE = bool(int(os.environ.get("GAT_TRACE", "0")))

# --------------------------------------------------------------------------
# custom DVE op: out = leakyrelu(in0*s0 + in1*s1) with slope (1 - imm2)
#   z = Src0*C0 + Src1*C1 ; out = z - min(z, 0)*C2      (imm2 = 0.8 -> 0.2)
# --------------------------------------------------------------------------


def _comb_ref(in0, in1, s0, s1, imm2):
    z = in0.astype(np.float32) * s0 + in1.astype(np.float32) * s1
    return (z - np.minimum(z, 0.0) * imm2).astype(np.float32)


@functools.cache
def _register_comb_op():
    import concourse.dve_ops as dve_ops
    from concourse.dve_spec import Spec, Src0, Src1, C0, C1, C2, Zero, minn, lower
    from concourse.dve_uop import DveOpSpec

    name = "GAT_COMB_LRELU"
    if name in dve_ops._SUB_OPCODE_FOR_NAME:
        return next(op for op in dve_ops.OPS if op.name == name)

    z = Src0 * C0 + Src1 * C1
    spec = Spec(body=z - minn(z, Zero) * C2, reference=_comb_ref)

    row = dve_ops._CUSTOM_DVE_ROW_BASE + len(dve_ops.OPS)
    assert row < 0x20
    shas = {}
    for ver in ("v3", "v4"):
        s = DveOpSpec(name=name, opcode=row, uops=lower(spec, ver=ver), rd1_en=True)
        shas[ver] = s.sha(ver)
    op = dve_ops.DveOp(name, spec, subdim=False, uops_sha=shas)
    dve_ops.OPS.append(op)
    dve_ops.CUSTOM_DVE_SPECS[name] = spec
    dve_ops._SUB_OPCODE_FOR_NAME[name] = row
    return op


def _elu_tail_ref(in0, in1, s0, s1, imm2):
    return (np.maximum(in0.astype(np.float32), 0.0) + in1 - 1.0).astype(np.float32)


@functools.cache
def _register_elu_tail_op():
    import concourse.dve_ops as dve_ops
    from concourse.dve_spec import Spec, Src0, Src1, Zero, One, maxx, lower
    from concourse.dve_uop import DveOpSpec

    name = "GAT_ELU_TAIL"
    if name in dve_ops._SUB_OPCODE_FOR_NAME:
        return next(op for op in dve_ops.OPS if op.name == name)

    spec = Spec(body=maxx(Src0, Zero) + Src1 - One, reference=_elu_tail_ref)
    row = dve_ops._CUSTOM_DVE_ROW_BASE + len(dve_ops.OPS)
    assert row < 0x20
    shas = {}
    for ver in ("v3", "v4"):
        sp = DveOpSpec(name=name, opcode=row, uops=lower(spec, ver=ver), rd1_en=True)
        shas[ver] = sp.sha(ver)
    op = dve_ops.DveOp(name, spec, subdim=False, uops_sha=shas)
    dve_ops.OPS.append(op)
    dve_ops.CUSTOM_DVE_SPECS[name] = spec
    dve_ops._SUB_OPCODE_FOR_NAME[name] = row
    return op


def _ap_with(ap: bass.AP, dims, offset_elems=0):
    """Clone `ap` with explicit [step, count] dims (element units).

    dims[0] is the partition dim; pass None to keep the AP's own.
    """
    dims = [list(ap.ap[0]) if d is None else list(d) for d in dims]
    return dataclasses.replace(ap, ap=dims, offset=ap.offset + offset_elems)


# --------------------------------------------------------------------------
# launch 1: xsum over features (per core: x slice [PN, 128] -> xsum [PN])
# --------------------------------------------------------------------------


@functools.cache
def _build_nc1():
    nc = bacc.Bacc("TRN2", target_bir_lowering=False, debug=False)
    x_d = nc.dram_tensor("x", [PN, F_IN], F32, kind="ExternalInput")
    xs_d = nc.dram_tensor("xsum", [PN], F32, kind="ExternalOutput")

    x_ap = x_d.ap().rearrange("(p b) f -> p b f", p=128)       # [128, 98, 128]
    xs_ap = xs_d.ap().rearrange("(p b) -> p b", p=128)         # [128, 98]

    CH = 14
    BPC = B // CH
    with tile.TileContext(nc) as tc:
        with (
            tc.tile_pool(name="xin", bufs=3) as xin_pool,
            tc.tile_pool(name="xs", bufs=1) as xs_pool,
        ):
            xs_t = xs_pool.tile([128, B], F32)
            for ch in range(CH):
                xt = xin_pool.tile([128, BPC, F_IN], F32)
                nc.sync.dma_start(out=xt[:], in_=x_ap[:, ch * BPC:(ch + 1) * BPC, :])
                nc.vector.tensor_reduce(
                    out=xs_t[:, ch * BPC:(ch + 1) * BPC],
                    in_=xt[:],
                    axis=mybir.AxisListType.X,
                    op=ALU.add,
                )
            nc.sync.dma_start(out=xs_ap, in_=xs_t[:])
    nc.compile()
    return nc


# --------------------------------------------------------------------------
# launch 2: the GAT edge math on dense degree-banded grids
# --------------------------------------------------------------------------


@functools.cache
def _build_nc2(w0: int, w1: int, nb1: int):
    comb = _register_comb_op()
    elu_tail = _register_elu_tail_op()
    FB0 = B * w0
    FB1 = nb1 * w1
    PDT = BF16 if USE_BF16_PROD else F32

    nc = bacc.Bacc("TRN2", target_bir_lowering=False, debug=False, num_devices=NCORES)
    # all grid inputs arrive pre-swizzled: [128 partitions, per-partition data]
    v_d = nc.dram_tensor("v", [128, B], F32, kind="ExternalInput")
    pc_d = nc.dram_tensor("pc", [128, B], F32, kind="ExternalInput")
    # br = [cs(8) cd(8) -M(8) exp(-M)(8)]: weight-derived scalars, host-folded
    br_d = nc.dram_tensor("br", [1, 32], F32, kind="ExternalInput")
    ws_d = nc.dram_tensor("wsvec", [1, H * D], F32, kind="ExternalInput")
    b_d = nc.dram_tensor("bvec", [1, H * D], F32, kind="ExternalInput")
    u0_d = nc.dram_tensor("U0", [128, FB0], F32, kind="ExternalInput")
    u016_d = nc.dram_tensor("U0_16", [128, FB0], PDT, kind="ExternalInput")
    if nb1:
        u1_d = nc.dram_tensor("U1", [128, FB1], F32, kind="ExternalInput")
        u116_d = nc.dram_tensor("U1_16", [128, FB1], PDT, kind="ExternalInput")
    out_d = nc.dram_tensor("out", [128, B * H * D], F32, kind="ExternalOutput")

    zin_d = nc.dram_tensor("zin", [1, H], F32)
    zout_d = nc.dram_tensor("zout", [NCORES, H], F32, addr_space="Shared")

    with tile.TileContext(nc) as tc:
        with (
            tc.tile_pool(name="singles", bufs=1) as singles,
            tc.tile_pool(name="work", bufs=2) as work,
            tc.tile_pool(name="small", bufs=2) as small,
            tc.tile_pool(name="psum", bufs=2, space="PSUM") as psum,
        ):
            # ---- small loads first ----
            V1 = singles.tile([128, B], F32)
            nc.sync.dma_start(out=V1[:], in_=v_d.ap())
            PC1 = singles.tile([128, B], F32)
            nc.sync.dma_start(out=PC1[:], in_=pc_d.ap())
            BR = singles.tile([128, 32], F32)
            nc.sync.dma_start(out=BR[:], in_=_ap_with(br_d.ap(), [[0, 128], [1, 32]]))
            ws_row = singles.tile([1, H * D], F32)
            nc.sync.dma_start(out=ws_row[:], in_=ws_d.ap())
            bias_t = singles.tile([1, H * D], F32)
            nc.sync.dma_start(out=bias_t[:], in_=b_d.ap())
            # ---- big grid loads ----
            U0 = singles.tile([128, FB0], F32)
            nc.sync.dma_start(out=U0[:], in_=u0_d.ap())
            if nb1:
                U1 = singles.tile([128, FB1], F32)
                nc.sync.dma_start(out=U1[:], in_=u1_d.ap())
            U016 = singles.tile([128, FB0], PDT)
            nc.sync.dma_start(out=U016[:], in_=u016_d.ap())
            if nb1:
                U116 = singles.tile([128, FB1], PDT)
                nc.sync.dma_start(out=U116[:], in_=u116_d.ap())

            ones_col = singles.tile([128, 1], F32)
            nc.vector.memset(ones_col[:], 1.0)
            ones_row = singles.tile([1, 128], F32)
            nc.vector.memset(ones_row[:], 1.0)

            # ---- V broadcast along occ for both bands ----
            VB0 = singles.tile([128, FB0], F32)
            nc.vector.tensor_copy(
                out=VB0[:], in_=_ap_with(V1[:], [None, [1, B], [0, w0]]),
            )
            if nb1:
                VB1 = singles.tile([128, FB1], F32)
                nc.vector.tensor_copy(
                    out=VB1[:], in_=_ap_with(V1[:], [None, [1, nb1], [0, w1]]),
                )

            # ---- phase 1: exp grids for all heads (Z ready early) ----
            zgrid = singles.tile([128, 2 * H], F32)   # [band0 | band1]
            T = singles.tile([128, H, B], F32)
            p0s = singles.tile([128, H, FB0], PDT, name="p0s")
            p1s = singles.tile([128, H, FB1], PDT, name="p1s") if nb1 else None

            for h in range(H):
                w0_t = work.tile([128, FB0], F32, tag="w0", name=f"w0t{h}")
                nc.vector._custom_dve(
                    comb, out=w0_t[:], in0=U0[:], in1=VB0[:],
                    s0=BR[:, h:h + 1], s1=BR[:, 8 + h:9 + h], imm2=0.8,
                )
                nc.scalar.activation(
                    p0s[:, h, :], w0_t[:], ACTF.Exp,
                    bias=BR[:, 16 + h:17 + h], scale=1.0,
                    accum_out=zgrid[:, h:h + 1],
                )
                if nb1:
                    w1_t = work.tile([128, FB1], F32, tag="w1", name=f"w1t{h}")
                    nc.vector._custom_dve(
                        comb, out=w1_t[:], in0=U1[:], in1=VB1[:],
                        s0=BR[:, h:h + 1], s1=BR[:, 8 + h:9 + h], imm2=0.8,
                    )
                    nc.scalar.activation(
                        p1s[:, h, :], w1_t[:], ACTF.Exp,
                        bias=BR[:, 16 + h:17 + h], scale=1.0,
                        accum_out=zgrid[:, H + h:H + h + 1],
                    )

            # ---- batched pad correction ----
            # padz[:, h] = em[h] * sum_b pc[b] * exp(lrelu(cd[h]*v[b]))
            padz = singles.tile([128, H], F32)
            cdv = small.tile([128, H, B], F32, tag="cdv")
            nc.vector.tensor_tensor(
                out=cdv[:],
                in0=_ap_with(V1[:], [None, [0, H], [1, B]]),
                in1=_ap_with(BR[:], [None, [1, H], [0, B]], offset_elems=8),
                op=ALU.mult,
            )
            cdm = small.tile([128, H, B], F32, tag="cdm")
            nc.vector.tensor_scalar_min(cdm[:], cdv[:], 0.0)
            nc.vector.tensor_scalar(
                out=cdm[:], in0=cdm[:], scalar1=-0.8, scalar2=None, op0=ALU.mult,
            )
            nc.vector.tensor_add(cdm[:], cdm[:], cdv[:])
            pexp = small.tile([128, H, B], F32, tag="pexp")
            nc.scalar.activation(pexp[:], cdm[:], ACTF.Exp)
            pw = small.tile([128, H, B], F32, tag="pw")
            nc.vector.tensor_tensor(
                out=pw[:],
                in0=_ap_with(PC1[:], [None, [0, H], [1, B]]),
                in1=_ap_with(BR[:], [None, [1, H], [0, B]], offset_elems=24),
                op=ALU.mult,
            )
            nc.vector.tensor_mul(pexp[:], pexp[:], pw[:])
            nc.vector.tensor_reduce(
                out=padz[:], in_=pexp[:], axis=mybir.AxisListType.X, op=ALU.add,
            )

            # ---- Z partials out + AllReduce (overlaps phase 2) ----
            zv = singles.tile([128, H], F32)
            if nb1:
                nc.vector.tensor_add(zv[:], zgrid[:, 0:H], zgrid[:, H:2 * H])
                nc.vector.tensor_sub(zv[:], zv[:], padz[:])
            else:
                nc.vector.tensor_sub(zv[:], zgrid[:, 0:H], padz[:])
            psum_z = psum.tile([1, H], F32)
            nc.tensor.matmul(psum_z[:], ones_col[:], zv[:])
            zr = singles.tile([1, H], F32)
            nc.vector.tensor_copy(zr[:], psum_z[:])
            nc.sync.dma_start(out=zin_d.ap(), in_=zr[:])
            nc.gpsimd.collective_compute(
                "AllGather",
                ALU.bypass,
                replica_groups=[list(range(NCORES))],
                ins=[zin_d.ap()],
                outs=[zout_d.ap()],
            )

            # ---- phase 2: t = sum_j u * p per block (overlaps the AR) ----
            for h in range(H):
                prod0 = work.tile([128, FB0], PDT, tag="prod0", name=f"pr0t{h}")
                nc.vector.tensor_mul(prod0[:], U016[:], p0s[:, h, :])
                nc.vector.tensor_reduce(
                    out=T[:, h, :],
                    in_=prod0[:].rearrange("p (b j) -> p b j", b=B),
                    axis=mybir.AxisListType.X,
                    op=ALU.add,
                )
                if nb1:
                    prod1 = work.tile([128, FB1], PDT, tag="prod1", name=f"pr1t{h}")
                    nc.vector.tensor_mul(prod1[:], U116[:], p1s[:, h, :])
                    t1 = small.tile([128, nb1], F32, tag="t1", name=f"t1t{h}")
                    nc.vector.tensor_reduce(
                        out=t1[:],
                        in_=prod1[:].rearrange("p (b j) -> p b j", b=nb1),
                        axis=mybir.AxisListType.X,
                        op=ALU.add,
                    )
                    nc.vector.tensor_add(T[:, h, 0:nb1], T[:, h, 0:nb1], t1[:])

            zgat = singles.tile([NCORES, H], F32)
            nc.sync.dma_start(out=zgat[:], in_=zout_d.ap())
            psum_zt = psum.tile([1, H], F32, name="psum_zt")
            nc.tensor.matmul(psum_zt[:], ones_col[0:NCORES, :], zgat[:])
            zr2 = singles.tile([1, H], F32)
            nc.vector.tensor_copy(zr2[:], psum_zt[:])
            zinv = singles.tile([1, H], F32)
            nc.vector.reciprocal(zinv[:], zr2[:])

            # wz[h*8+d] = ws[h,d]*zinv[h]; pack [wz(64) bias(64)], bcast via PE
            wzrow = singles.tile([1, 128], F32)
            nc.vector.tensor_tensor(
                out=wzrow[:, 0:64].rearrange("p (h d) -> p h d", h=H),
                in0=ws_row[:].rearrange("p (h d) -> p h d", h=H),
                in1=_ap_with(zinv[:], [None, [1, H], [0, D]]),
                op=ALU.mult,
            )
            nc.vector.tensor_copy(wzrow[:, 64:128], bias_t[:])
            psum_wb = psum.tile([128, 128], F32)
            nc.tensor.matmul(psum_wb[:], ones_row[:], wzrow[:])
            WB = singles.tile([128, 128], F32)
            nc.vector.tensor_copy(WB[:], psum_wb[:])

            # ---- epilogue: out = elu(t*wz + bias), 7 chunks of 14 blocks ----
            # engine split: DVE (y-mult, max-add), ACT (relu(-s), exp(-r)),
            # GPSIMD (bias-add, final add)
            CB = 14
            t_pstep = T[:].ap[0][0]
            wb_pstep = WB[:].ap[0][0]
            out_ap = out_d.ap().rearrange("p (b c) -> p b c", c=H * D)
            with (
                tc.tile_pool(name="epi", bufs=2) as epi,
            ):
                for ch in range(B // CB):
                    y_t = epi.tile([128, CB, H * D], F32, tag="y")
                    # y[p, b, h*8+d] = T[p, h, ch*CB+b] * WB[p, h*8+d]
                    nc.vector.tensor_tensor(
                        out=y_t[:],
                        in0=_ap_with(T[:], [[t_pstep, 128], [1, CB], [B, H], [0, D]],
                                     offset_elems=ch * CB),
                        in1=_ap_with(WB[:], [[wb_pstep, 128], [0, CB], [1, H * D]]),
                        op=ALU.mult,
                    )
                    s_t = epi.tile([128, CB, H * D], F32, tag="s")
                    eng = nc.gpsimd if ch % 2 == 0 else nc.vector
                    eng.tensor_tensor(
                        out=s_t[:],
                        in0=y_t[:],
                        in1=_ap_with(WB[:], [[wb_pstep, 128], [0, CB], [1, H * D]],
                                     offset_elems=64),
                        op=ALU.add,
                    )
                    # r = relu(-s) = -min(s,0);  q = exp(-r) = exp(min(s,0))
                    r_t = epi.tile([128, CB, H * D], F32, tag="r")
                    nc.scalar.activation(r_t[:], s_t[:], ACTF.Relu, scale=-1.0)
                    q_t = epi.tile([128, CB, H * D], F32, tag="q")
                    nc.scalar.activation(q_t[:], r_t[:], ACTF.Exp, scale=-1.0)
                    o_t = epi.tile([128, CB, H * D], F32, tag="o")
                    # elu(s) = max(s,0) + exp(min(s,0)) - 1
                    nc.vector._custom_dve(
                        elu_tail, out=o_t[:].rearrange("p b c -> p (b c)"),
                        in0=s_t[:].rearrange("p b c -> p (b c)"),
                        in1=q_t[:].rearrange("p b c -> p (b c)"),
                    )
                    nc.sync.dma_start(
                        out=out_ap[:, ch * CB:(ch + 1) * CB, :], in_=o_t[:],
                    )
    nc.compile()
    return nc


# --------------------------------------------------------------------------
# host orchestration
# --------------------------------------------------------------------------


def _swz(arr):
    """[PN, ...] row-major (block-major node ids: ln = b*128 + p) ->
    device layout [128, B, ...] flattened per partition."""
    rest = arr.shape[1:]
    return np.ascontiguousarray(
        arr.reshape(B, 128, *rest).transpose(1, 0, *range(2, 2 + len(rest)))
    ).reshape(128, -1)


def kernel(x, edge_index, W, a, bias):
    x = np.ascontiguousarray(np.asarray(x), dtype=np.float32)
    ei = np.asarray(edge_index)
    src = ei[0].astype(np.int64)
    dst = ei[1].astype(np.int64)
    W = np.asarray(W, dtype=np.float32)
    a = np.asarray(a, dtype=np.float32)
    bias = np.asarray(bias, dtype=np.float32)

    cores = list(range(NCORES))

    # ---- launch 1: xsum ----
    nc1 = _build_nc1()
    in1 = []
    for c in cores:
        xp = np.zeros((PN, F_IN), dtype=np.float32)
        xp[:NPC] = x[c * NPC:(c + 1) * NPC]
        in1.append({"x": xp})
    r1 = run_bass_kernel_spmd(nc1, in1, cores, trace=_TRACE)
    xsum = np.concatenate([r1.results[c]["xsum"][:NPC] for c in cores])
    LAST_STATS["launch1_ns"] = r1.exec_time_ns

    gmax = float(np.abs(xsum).max())

    # ---- host: degree-sorted banded grids (index work + one gather) ----
    deg = np.bincount(dst, minlength=N)
    degc = deg.reshape(NCORES, NPC)
    dmax = int(deg.max())

    # pick band split minimizing total slots
    best = None
    for cand in range(8, dmax + 1):
        cnt = int((degc > cand).sum(1).max())
        nb = -(-cnt // 128) if cnt else 0
        slots = PN * cand + nb * 128 * (dmax - cand)
        if best is None or slots < best[0]:
            best = (slots, cand, nb)
    _, W0, nb1 = best
    W1 = dmax - W0 if nb1 else 0

    # per-core degree-descending node order
    deg_pad = np.zeros((NCORES, PN), dtype=np.int64)
    deg_pad[:, :NPC] = degc
    perm = np.argsort(-deg_pad, axis=1, kind="stable")       # [NC, PN]
    inv_perm = np.argsort(perm, axis=1)                      # orig ln -> sorted row

    # per-edge target slots
    order = np.argsort(dst, kind="stable")
    sdst = dst[order]
    usrc = xsum[src[order]]
    starts = np.concatenate([[0], np.cumsum(deg)[:-1]])
    occ = np.arange(E, dtype=np.int64) - starts[sdst]
    core_idx = sdst // NPC
    row = inv_perm[core_idx, sdst % NPC]                     # sorted row id

    U0_all = np.zeros((NCORES, PN, W0), dtype=np.float32)
    m0 = occ < W0
    U0_all[core_idx[m0], row[m0], occ[m0]] = usrc[m0]
    if nb1:
        U1_all = np.zeros((NCORES, nb1 * 128, W1), dtype=np.float32)
        m1 = ~m0
        U1_all[core_idx[m1], row[m1], occ[m1] - W0] = usrc[m1]

    cap = np.full(PN, W0, dtype=np.float32)
    cap[:nb1 * 128] += W1
    sdeg = np.take_along_axis(deg_pad, perm, axis=1).astype(np.float32)
    pc_all = cap[None, :] - sdeg                             # [NC, PN]

    v_all = np.zeros((NCORES, PN), dtype=np.float32)
    v_all[:, :NPC] = xsum.reshape(NCORES, NPC)
    v_all = np.take_along_axis(v_all, perm, axis=1)

    pdt = ml_dtypes.bfloat16 if USE_BF16_PROD else np.float32

    # weight-derived scalars (pure weight preprocessing)
    wsum = W.sum(1)                                 # [H, D]
    cs = (wsum * a[:, :D, 0]).sum(1)                # [H]
    cd = (wsum * a[:, D:, 0]).sum(1)
    M = (np.abs(cs) + np.abs(cd)) * gmax - 40.0
    brvec = np.concatenate([cs, cd, -M, np.exp(-M)]).astype(np.float32)[None, :]
    wsvec = np.ascontiguousarray(wsum.reshape(1, H * D)).astype(np.float32)
    bvec = np.ascontiguousarray(bias.reshape(1, H * D))

    # ---- launch 2 ----
    nc2 = _build_nc2(W0, W1, nb1)
    in2 = []
    for c in cores:
        m = {
            "v": _swz(v_all[c][:, None]),
            "pc": _swz(pc_all[c][:, None]),
            "br": brvec,
            "wsvec": wsvec,
            "bvec": bvec,
            "U0": _swz(U0_all[c]),
        }
        m["U0_16"] = m["U0"].astype(pdt)
        if nb1:
            u1 = np.ascontiguousarray(
                U1_all[c].reshape(nb1, 128, W1).transpose(1, 0, 2)
            ).reshape(128, -1)
            m["U1"] = u1
            m["U1_16"] = u1.astype(pdt)
        in2.append(m)
    r2 = run_bass_kernel_spmd(nc2, in2, cores, trace=_TRACE)
    LAST_STATS["launch2_ns"] = r2.exec_time_ns

    out = np.empty((N, H * D), dtype=np.float32)
    for c in cores:
        o = r2.results[c]["out"].reshape(128, B, H * D).transpose(1, 0, 2)
        o = o.reshape(PN, H * D)          # rows in sorted order
        sel = perm[c] < NPC
        out[c * NPC + perm[c][sel]] = o[sel]
    return out
